# revision 1
# baseline (speedup 1.0000x reference)
"""Causal multi-head attention (B=2, L=2048, D=1024, H=16, Dh=64) on 8 TRN2
NeuronCores.

Sharding: data-parallel over B (2 groups of 4 cores), tensor-parallel over H
within a group (4 heads per core). Each core computes QKV projections for its
heads, full causal attention per head (flash-style, scores kept transposed so
no on-chip transposes are needed), and a partial output projection
y_c = sum_h o_h @ Wout_h. The host sums the 4 partials per batch.

Per-core layout choices:
  - x is pre-transposed on the host (xT [D, L]) so the QKV contraction dim D
    lands on SBUF partitions directly.
  - q, k are produced transposed (qT/kT [e, L]) so the scores matmul
    ST = K_h @ Q_h^T contracts over Dh on partitions; softmax runs on ST
    tiles [k=128, q=512] with the reduction (sum over k) folded into the
    P@V matmul via a ones-row appended to V (lhsT [128, 65]; row 64 of the
    PSUM result is the softmax denominator).
  - Projections run in float32r (TF32-class, 1 cycle/row at N>=256);
    the attention matmuls (scores, P@V) run in bf16 with f32 PSUM
    accumulation, which makes their weight loads FWL-fast.
"""

import numpy as np

import concourse.bass as bass
import concourse.mybir as mybir
import concourse.tile as tile
from concourse import bacc
from concourse.bass_utils import run_bass_kernel_spmd

F32 = mybir.dt.float32
F32R = mybir.dt.float32r
BF16 = mybir.dt.bfloat16
EXP = mybir.ActivationFunctionType.Exp
MULT = mybir.AluOpType.mult

B, L, D, H = 2, 2048, 1024, 16
Dh = D // H
NCORES = 8
NH = 4            # heads per core
EL = NH * Dh      # local head dims = 256
P = 128
NQ = 512          # q-chunk width (scores free dim)
QC = L // NQ      # 4 q-chunks
DC = D // P       # 8 contraction chunks for projections
LC = 4            # xT l-chunks for QKV
NL = L // LC      # 512


def build():
    nc = bacc.Bacc("TRN2", target_bir_lowering=False, debug=False,
                   num_devices=NCORES)

    xT = nc.dram_tensor("xT", [D, L], BF16, kind="ExternalInput")
    wq = nc.dram_tensor("wq", [D, EL], BF16, kind="ExternalInput")
    wk = nc.dram_tensor("wk", [D, EL], BF16, kind="ExternalInput")
    wv = nc.dram_tensor("wv", [D, EL], BF16, kind="ExternalInput")
    wout = nc.dram_tensor("wout", [EL, D], BF16, kind="ExternalInput")
    masks = nc.dram_tensor("masks", [P, P], BF16, kind="ExternalInput")
    out = nc.dram_tensor("out", [L, D], F32, kind="ExternalOutput")

    scale = 1.0 / np.sqrt(Dh)

    with tile.TileContext(nc) as tc:
        with (
            tc.tile_pool(name="const", bufs=1) as cpool,
            tc.tile_pool(name="xt", bufs=2) as xpool,
            tc.tile_pool(name="pt", bufs=6) as ptpool,
            tc.tile_pool(name="work", bufs=3) as wpool,
            tc.tile_pool(name="norm", bufs=8) as npool,
            tc.tile_pool(name="dram", bufs=8, space="DRAM") as dpool,
            tc.tile_pool(name="mm", bufs=2, space="PSUM") as mm_ps,
            tc.tile_pool(name="st", bufs=2, space="PSUM") as st_ps,
            tc.tile_pool(name="pv", bufs=2, space="PSUM") as pv_ps,
        ):
            # ---- persistent SBUF tensors ----
            wq_sb = cpool.tile([P, DC, EL], BF16, tag="wq")
            wk_sb = cpool.tile([P, DC, EL], BF16, tag="wk")
            wv_sb = cpool.tile([P, DC, EL], BF16, tag="wv")
            wout_sb = cpool.tile([P, EL // P, D], BF16, tag="wout")
            mask_sb = cpool.tile([P, P], BF16, tag="mask")
            qT_sb = cpool.tile([P, EL // P, L], BF16, tag="qT")
            kT_sb = cpool.tile([P, EL // P, L], BF16, tag="kT")
            vext_sb = cpool.tile([P, L // P, NH, Dh + 1], BF16, tag="vext")
            oT_sb = cpool.tile([P, EL // P, L], BF16, tag="oT")
            ones_f32 = cpool.tile([P, P], F32, tag="onesf")
            ones_sb = cpool.tile([P, P], F32R, tag="ones")

            # DMA order matters at startup: the first QKV matmul group needs
            # wq + the first xT chunk; everything else can trickle in behind
            xT_r = xT.ap().rearrange("(o p) l -> p o l", p=P)
            wq_r = wq.ap().rearrange("(o p) e -> p o e", p=P)
            xt0 = xpool.tile([P, DC, NL], BF16, tag="xt", name="xt0")
            # split the first loads across several DMA queues so the first
            # matmul group isn't gated on two single-queue transfers
            for dc in range(0, DC, 2):
                nc.sync.dma_start(wq_sb[:, dc:dc + 2, :], wq_r[:, dc:dc + 2, :])
                nc.sync.dma_start(xt0[:, dc:dc + 2, :], xT_r[:, dc:dc + 2, 0:NL])
            nc.sync.dma_start(
                wk_sb[:], wk.ap().rearrange("(o p) e -> p o e", p=P))
            nc.sync.dma_start(
                wv_sb[:], wv.ap().rearrange("(o p) e -> p o e", p=P))
            nc.sync.dma_start(
                wout_sb[:], wout.ap().rearrange("(o p) d -> p o d", p=P))
            nc.sync.dma_start(mask_sb[:], masks[:, :])

            nc.vector.memset(ones_f32[:], 1.0)
            nc.vector.tensor_copy(out=ones_sb[:], in_=ones_f32[:])
            # ones column of vext (the softmax-denominator row of P@V)
            nc.vector.tensor_copy(
                out=vext_sb[:, :, :, Dh],
                in_=ones_f32[:, 0:L // P * NH].rearrange("p (a b) -> p a b", a=L // P),
            )

            def emit_qkv(lc):
                if lc == 0:
                    xt = xt0
                else:
                    xt = xpool.tile([P, DC, NL], BF16, tag="xt",
                                    name=f"xt{lc}")
                    for dc in range(0, DC, 2):
                        nc.sync.dma_start(
                            xt[:, dc:dc + 2, :],
                            xT_r[:, dc:dc + 2, lc * NL:(lc + 1) * NL])

                for w_sb, dst in ((wq_sb, qT_sb), (wk_sb, kT_sb)):
                    for ec in range(EL // P):
                        ps = mm_ps.tile([P, NQ], F32, tag="mm",
                                        name=f"qk_{lc}_{ec}")
                        for dc in range(DC):
                            nc.tensor.matmul(
                                ps[:],
                                w_sb[:, dc, ec * P:(ec + 1) * P],
                                xt[:, dc, :],
                                start=(dc == 0), stop=(dc == DC - 1),
                            )
                        nc.vector.tensor_copy(
                            out=dst[:, ec, lc * NL:(lc + 1) * NL], in_=ps[:])

                for lt in range(NL // P):
                    lo = lc * (NL // P) + lt
                    ps = mm_ps.tile([P, EL], F32, tag="mm",
                                    name=f"v_{lc}_{lt}")
                    for dc in range(DC):
                        nc.tensor.matmul(
                            ps[:],
                            xt[:, dc, lt * P:(lt + 1) * P],
                            wv_sb[:, dc, :],
                            start=(dc == 0), stop=(dc == DC - 1),
                        )
                    nc.vector.tensor_copy(
                        out=vext_sb[:, lo, :, 0:Dh],
                        in_=ps[:].rearrange("p (h e) -> p h e", h=NH),
                    )

            norm_state = {}        # (qc, h) -> (ot_un, rr)

            def emit_attn_pair(qc, pair):
                nk = 4 * (qc + 1)          # causal k-chunks of 128
                if True:
                    heads = (2 * pair, 2 * pair + 1)
                    pts = {}               # (h, ki) -> pt tile
                    pvs = {}               # h -> accumulating PSUM tile
                    for ki in range(nk):
                        j = ki - 4 * qc    # >=0 on diagonal-crossing tiles
                        # both heads' score tiles share one 2-bank PSUM tile
                        # so a single EXP covers the pair
                        stp = st_ps.tile([P, 2, NQ], F32, tag="st",
                                         name=f"st_{qc}_{pair}_{ki}")
                        ptp = ptpool.tile([P, 2, NQ], BF16, tag="pt",
                                          name=f"pt_{qc}_{pair}_{ki}")
                        for idx, h in enumerate(heads):
                            hp = (h % 2) * 64
                            ec = h // 2
                            nc.tensor.matmul(
                                stp[:, idx, :],
                                kT_sb[hp:hp + 64, ec, ki * P:(ki + 1) * P],
                                qT_sb[hp:hp + 64, ec, qc * NQ:(qc + 1) * NQ],
                                start=True, stop=True,
                            )
                            pts[(h, ki)] = ptp[:, idx, :]
                        if j < 0:
                            nc.scalar.activation(out=ptp[:], in_=stp[:],
                                                 func=EXP, scale=scale)
                        else:
                            # columns left of the diagonal block are fully
                            # masked; the diagonal 128-block needs the
                            # triangular mask; the rest is unmasked
                            if j > 0:
                                nc.gpsimd.memset(ptp[:, :, 0:P * j], 0.0)
                            nc.scalar.activation(
                                out=ptp[:, :, P * j:], in_=stp[:, :, P * j:],
                                func=EXP, scale=scale)
                            for idx in range(2):
                                nc.gpsimd.tensor_tensor(
                                    out=ptp[:, idx, P * j:P * (j + 1)],
                                    in0=ptp[:, idx, P * j:P * (j + 1)],
                                    in1=mask_sb[:, :], op=MULT)
                        # P@V runs one ki behind the scores so the in-order
                        # PE stream never waits on the exp of the current ki
                        if ki >= 1:
                            for h in heads:
                                if ki == 1:
                                    pvs[h] = pv_ps.tile([Dh + 1, NQ], F32,
                                                        name=f"po_{qc}_{h}",
                                                        tag="pv")
                                nc.tensor.matmul(
                                    pvs[h][:],
                                    vext_sb[:, ki - 1, h, :],
                                    pts.pop((h, ki - 1)),
                                    start=(ki == 1), stop=False,
                                )
                    for h in heads:
                        nc.tensor.matmul(
                            pvs[h][:],
                            vext_sb[:, nk - 1, h, :],
                            pts.pop((h, nk - 1)),
                            start=False, stop=True,
                        )

                    for h in heads:
                        po = pvs[h]
                        # norm front half: evacuate PSUM and kick off the
                        # reciprocal chain (no PE instructions, so the PE
                        # stream never waits on it here)
                        ot_un = npool.tile([64, NQ], F32, tag="otun",
                                           name=f"otun_{qc}_{h}")
                        nc.any.tensor_copy(out=ot_un[:], in_=po[0:64, :])
                        rsum = npool.tile([P, NQ], F32, tag="rsum",
                                          name=f"rsum_{qc}_{h}")
                        nc.vector.tensor_copy(out=rsum[64:65, :],
                                              in_=po[64:65, :])
                        rr_f = npool.tile([P, NQ], F32, tag="rrf",
                                          name=f"rrf_{qc}_{h}")
                        if qc == QC - 1 and h >= 2:
                            # last pair: nothing overlaps the tail, so the
                            # single-op reciprocal's shorter latency beats
                            # the reshaped chain's throughput
                            nc.vector.reciprocal(rr_f[64:65, :],
                                                 rsum[64:65, :])
                        else:
                            # reshape the [1,512] rsum row to [64,8] via a
                            # DRAM bounce so the reciprocal uses 64 DVE lanes
                            dr1 = dpool.tile([NQ], F32,
                                             name=f"dr1_{qc}_{h}", tag="dr1")
                            nc.sync.dma_start(
                                dr1[:].rearrange("(a b) -> a b", a=1),
                                rsum[64:65, :])
                            r64 = npool.tile([64, NQ // 64], F32, tag="r64",
                                             name=f"r64_{qc}_{h}")
                            nc.sync.dma_start(
                                r64[:],
                                dr1[:].rearrange("(a b) -> a b", b=NQ // 64))
                            nc.vector.reciprocal(r64[:], r64[:])
                            dr2 = dpool.tile([NQ], F32,
                                             name=f"dr2_{qc}_{h}", tag="dr2")
                            nc.sync.dma_start(
                                dr2[:].rearrange("(a b) -> a b", b=NQ // 64),
                                r64[:])
                            nc.sync.dma_start(
                                rr_f[64:65, :],
                                dr2[:].rearrange("(a b) -> a b", a=1))
                        rr = npool.tile([P, NQ], F32R, tag="rr",
                                        name=f"rr_{qc}_{h}")
                        nc.vector.tensor_copy(out=rr[64:65, :],
                                              in_=rr_f[64:65, :])
                        norm_state[(qc, h)] = (ot_un, rr)

            def emit_norm_back(qc):
                for h in range(NH):
                    emit_norm_back_h(qc, h)

            def emit_norm_back_h(qc, h):
                # back half: broadcast the reciprocal row via a K=1 matmul
                # and scale; by the time the PE stream reaches these, the
                # recip chains have had a whole phase to complete
                if True:
                    hp = (h % 2) * 64
                    ec = h // 2
                    ot_un, rr = norm_state.pop((qc, h))
                    ps_bc = mm_ps.tile([64, NQ], F32, tag="mm",
                                       name=f"bc_{qc}_{h}")
                    nc.tensor.matmul(ps_bc[:], ones_sb[64:65, 0:64],
                                     rr[64:65, :], start=True, stop=True)
                    rs_sb = wpool.tile([64, NQ], F32, tag="rs")
                    nc.vector.tensor_copy(out=rs_sb[:], in_=ps_bc[:])
                    tmp = wpool.tile([64, NQ], BF16, tag="tmp")
                    nc.vector.tensor_tensor(out=tmp[:], in0=ot_un[:],
                                            in1=rs_sb[:], op=MULT)
                    nc.sync.dma_start(
                        oT_sb[hp:hp + 64, ec, qc * NQ:(qc + 1) * NQ],
                        tmp[:])

            def emit_proj(qc):
                # y = oT^T @ wout (partial over heads) for this q-chunk's rows
                for lt in range(4 * qc, 4 * (qc + 1)):
                    emit_proj_lt(lt)

            def emit_proj_lt(lt, pool=None, tag="mm"):
                if True:
                    for do in range(D // NQ):
                        ps = (pool or mm_ps).tile([P, NQ], F32, tag=tag,
                                                  name=f"y_{lt}_{do}")
                        for ec in range(EL // P):
                            nc.tensor.matmul(
                                ps[:],
                                oT_sb[:, ec, lt * P:(lt + 1) * P],
                                wout_sb[:, ec, do * NQ:(do + 1) * NQ],
                                start=(ec == 0), stop=(ec == EL // P - 1),
                            )
                        y_sb = wpool.tile([P, NQ], F32, tag="y")
                        nc.any.tensor_copy(out=y_sb[:], in_=ps[:])
                        nc.sync.dma_start(
                            out.ap()[lt * P:(lt + 1) * P,
                                     do * NQ:(do + 1) * NQ],
                            y_sb[:])

            # software-pipelined phase interleave: attention for q-chunk
            # ph only needs QKV through l-chunk ph; norm back-halves run one
            # phase late and projections two phases late so nothing in the
            # (in-order) PE stream waits on a slow dependency chain
            for ph in range(QC):
                emit_qkv(ph)
                if ph >= 1:
                    emit_norm_back(ph - 1)
                emit_attn_pair(ph, 0)
                if ph >= 2:
                    emit_proj(ph - 2)
                emit_attn_pair(ph, 1)
            # epilogue: interleave the last norm back-halves with the
            # already-ready qc=2 projection rows so the PE stream never
            # parks on a fresh reciprocal chain
            # pair 0's reciprocal chains finished during the second half of
            # attn(3); its norm-backs go first, then the ready qc=2
            # projection rows cover pair 1's chain latency
            emit_norm_back_h(QC - 1, 0)
            emit_norm_back_h(QC - 1, 1)
            emit_proj(QC - 2)
            emit_norm_back_h(QC - 1, 2)
            emit_norm_back_h(QC - 1, 3)
            for i, lt in enumerate(range(12, 16)):
                if i % 2 == 0:
                    emit_proj_lt(lt, pool=st_ps, tag="st")
                else:
                    emit_proj_lt(lt)

    nc.compile()
    return nc


def _host_masks():
    k = np.arange(P)[:, None]
    q = np.arange(P)[None, :]
    return (k <= q).astype(np.float32)


def _shard(x, Wq, Wk, Wv, Wout):
    import ml_dtypes
    bf16 = ml_dtypes.bfloat16
    masks = _host_masks()
    in_maps = []
    for c in range(NCORES):
        b, g = c // NH, c % NH
        hs = slice(g * NH, (g + 1) * NH)
        in_maps.append({
            "xT": np.ascontiguousarray(x[b].T).astype(bf16),
            "wq": np.ascontiguousarray(Wq[:, hs, :].reshape(D, EL)).astype(bf16),
            "wk": np.ascontiguousarray(Wk[:, hs, :].reshape(D, EL)).astype(bf16),
            "wv": np.ascontiguousarray(Wv[:, hs, :].reshape(D, EL)).astype(bf16),
            "wout": np.ascontiguousarray(Wout[hs].reshape(EL, D)).astype(bf16),
            "masks": masks.astype(bf16),
        })
    return in_maps


_NC_CACHE = None


def _get_nc():
    global _NC_CACHE
    if _NC_CACHE is None:
        _NC_CACHE = build()
    return _NC_CACHE


def run(x, Wq, Wk, Wv, Wout, trace=False):
    nc = _get_nc()
    in_maps = _shard(np.asarray(x), np.asarray(Wq), np.asarray(Wk),
                     np.asarray(Wv), np.asarray(Wout))
    res = run_bass_kernel_spmd(nc, in_maps, core_ids=list(range(NCORES)),
                               trace=trace)
    parts = [res.results[c]["out"] for c in range(NCORES)]
    full = np.stack([
        parts[0] + parts[1] + parts[2] + parts[3],
        parts[4] + parts[5] + parts[6] + parts[7],
    ]).astype(np.float32)
    return full, res


def kernel(x, Wq, Wk, Wv, Wout):
    for _ in range(3):
        full, _ = run(x, Wq, Wk, Wv, Wout, trace=False)
        if np.isfinite(full).all():
            return full
    return full



# revision 2
# speedup vs baseline: 1.0185x; 1.0185x over previous
"""Causal multi-head attention (B=2, L=2048, D=1024, H=16, Dh=64) on 8 TRN2
NeuronCores.

Sharding: data-parallel over B (2 groups of 4 cores), tensor-parallel over H
within a group (4 heads per core). Each core computes QKV projections for its
heads, full causal attention per head (flash-style, scores kept transposed so
no on-chip transposes are needed), and a partial output projection
y_c = sum_h o_h @ Wout_h. The host sums the 4 partials per batch.

v2 restructure vs the original baseline:
  - Trapezoid streaming: scores + exp + P@V only touch columns right of the
    causal diagonal (per 128-wide k-tile), instead of memset-zeroing masked
    regions and streaming full 512-wide tiles.
  - Phase schedule: norm back-halves run at the top of the next phase (before
    attention, covering the qkv->attn dependency boundary); projections are
    split around attn(pair1); the epilogue splits the last projections by
    ec-half (ec0 only needs heads 0/1) to hide the final reciprocal chains.
  - Reciprocal of the softmax denominator: ScalarE Ln + Exp(scale=-1) for the
    last phase (2 instructions, low latency, same activation table set as the
    softmax Exp), DMA-bounce + reciprocal_approx_fast for earlier phases.
  - Engine balance: output-projection PSUM evacuations split ScalarE/DVE; the
    o*(1/sum) scale-mult runs on GpSimd; norm-path DMAs ride the gpsimd queue.
"""

import numpy as np

import concourse.bass as bass
import concourse.mybir as mybir
import concourse.tile as tile
from concourse import bacc
from concourse.bass_utils import run_bass_kernel_spmd

F32 = mybir.dt.float32
F32R = mybir.dt.float32r
BF16 = mybir.dt.bfloat16
EXP = mybir.ActivationFunctionType.Exp
LN = mybir.ActivationFunctionType.Ln
MULT = mybir.AluOpType.mult

B, L, D, H = 2, 2048, 1024, 16
Dh = D // H
NCORES = 8
NH = 4            # heads per core
EL = NH * Dh      # local head dims = 256
P = 128
NQ = 512          # q-chunk width (scores free dim)
QC = L // NQ      # 4 q-chunks
DC = D // P       # 8 contraction chunks for projections
LC = 4            # xT l-chunks for QKV
NL = L // LC      # 512


def build():
    nc = bacc.Bacc("TRN2", target_bir_lowering=False, debug=False,
                   num_devices=NCORES)

    xT = nc.dram_tensor("xT", [D, L], BF16, kind="ExternalInput")
    wq = nc.dram_tensor("wq", [D, EL], BF16, kind="ExternalInput")
    wk = nc.dram_tensor("wk", [D, EL], BF16, kind="ExternalInput")
    wv = nc.dram_tensor("wv", [D, EL], BF16, kind="ExternalInput")
    wout = nc.dram_tensor("wout", [EL, D], BF16, kind="ExternalInput")
    masks = nc.dram_tensor("masks", [P, P], BF16, kind="ExternalInput")
    out = nc.dram_tensor("out", [L, D], F32, kind="ExternalOutput")

    scale = 1.0 / np.sqrt(Dh)

    with tile.TileContext(nc) as tc:
        with (
            tc.tile_pool(name="const", bufs=1) as cpool,
            tc.tile_pool(name="xt", bufs=2) as xpool,
            tc.tile_pool(name="pt", bufs=6) as ptpool,
            tc.tile_pool(name="work", bufs=3) as wpool,
            tc.tile_pool(name="norm", bufs=8) as npool,
            tc.tile_pool(name="dram", bufs=8, space="DRAM") as dpool,
            tc.tile_pool(name="mm", bufs=2, space="PSUM") as mm_ps,
            tc.tile_pool(name="st", bufs=2, space="PSUM") as st_ps,
            tc.tile_pool(name="pv", bufs=2, space="PSUM") as pv_ps,
        ):
            # ---- persistent SBUF tensors ----
            wq_sb = cpool.tile([P, DC, EL], BF16, tag="wq")
            wk_sb = cpool.tile([P, DC, EL], BF16, tag="wk")
            wv_sb = cpool.tile([P, DC, EL], BF16, tag="wv")
            wout_sb = cpool.tile([P, EL // P, D], BF16, tag="wout")
            mask_sb = cpool.tile([P, P], BF16, tag="mask")
            qT_sb = cpool.tile([P, EL // P, L], BF16, tag="qT")
            kT_sb = cpool.tile([P, EL // P, L], BF16, tag="kT")
            vext_sb = cpool.tile([P, L // P, NH, Dh + 1], BF16, tag="vext")
            oT_sb = cpool.tile([P, EL // P, L], BF16, tag="oT")
            ones_f32 = cpool.tile([P, P], F32, tag="onesf")
            ones_sb = cpool.tile([P, P], F32R, tag="ones")

            # DMA order matters at startup: the first QKV matmul group needs
            # wq + the first xT chunk; everything else can trickle in behind
            xT_r = xT.ap().rearrange("(o p) l -> p o l", p=P)
            wq_r = wq.ap().rearrange("(o p) e -> p o e", p=P)
            xt0 = xpool.tile([P, DC, NL], BF16, tag="xt", name="xt0")
            for dc in range(0, DC, 2):
                nc.sync.dma_start(wq_sb[:, dc:dc + 2, :], wq_r[:, dc:dc + 2, :])
                nc.sync.dma_start(xt0[:, dc:dc + 2, :], xT_r[:, dc:dc + 2, 0:NL])
            nc.sync.dma_start(
                wk_sb[:], wk.ap().rearrange("(o p) e -> p o e", p=P))
            nc.sync.dma_start(
                wv_sb[:], wv.ap().rearrange("(o p) e -> p o e", p=P))
            nc.sync.dma_start(
                wout_sb[:], wout.ap().rearrange("(o p) d -> p o d", p=P))
            nc.sync.dma_start(mask_sb[:], masks[:, :])

            nc.vector.memset(ones_f32[:], 1.0)
            nc.vector.tensor_copy(out=ones_sb[:], in_=ones_f32[:])
            # ones column of vext (the softmax-denominator row of P@V)
            nc.vector.tensor_copy(
                out=vext_sb[:, :, :, Dh],
                in_=ones_f32[:, 0:L // P * NH].rearrange("p (a b) -> p a b", a=L // P),
            )

            def emit_qkv(lc):
                if lc == 0:
                    xt = xt0
                else:
                    xt = xpool.tile([P, DC, NL], BF16, tag="xt",
                                    name=f"xt{lc}")
                    for dc in range(0, DC, 2):
                        nc.sync.dma_start(
                            xt[:, dc:dc + 2, :],
                            xT_r[:, dc:dc + 2, lc * NL:(lc + 1) * NL])

                for w_sb, dst in ((wq_sb, qT_sb), (wk_sb, kT_sb)):
                    for ec in range(EL // P):
                        ps = mm_ps.tile([P, NQ], F32, tag="mm",
                                        name=f"qk_{lc}_{ec}")
                        for dc in range(DC):
                            nc.tensor.matmul(
                                ps[:],
                                w_sb[:, dc, ec * P:(ec + 1) * P],
                                xt[:, dc, :],
                                start=(dc == 0), stop=(dc == DC - 1),
                            )
                        nc.vector.tensor_copy(
                            out=dst[:, ec, lc * NL:(lc + 1) * NL], in_=ps[:])

                for lt in range(NL // P):
                    lo = lc * (NL // P) + lt
                    ps = mm_ps.tile([P, EL], F32, tag="mm",
                                    name=f"v_{lc}_{lt}")
                    for dc in range(DC):
                        nc.tensor.matmul(
                            ps[:],
                            xt[:, dc, lt * P:(lt + 1) * P],
                            wv_sb[:, dc, :],
                            start=(dc == 0), stop=(dc == DC - 1),
                        )
                    nc.vector.tensor_copy(
                        out=vext_sb[:, lo, :, 0:Dh],
                        in_=ps[:].rearrange("p (h e) -> p h e", h=NH),
                    )

            norm_state = {}        # (qc, h) -> (ot_un, rr)

            def emit_attn_pair(qc, pair):
                nk = 4 * (qc + 1)          # causal k-chunks of 128
                heads = (2 * pair, 2 * pair + 1)
                pts = {}               # (h, ki) -> (pt AP [P, NQ], j)
                pvs = {}               # h -> accumulating PSUM tile

                def emit_pv(h, ki):
                    ap, j = pts.pop((h, ki))
                    lo = P * j if j > 0 else 0
                    nc.tensor.matmul(
                        pvs[h][:, lo:],
                        vext_sb[:, ki, h, :],
                        ap[:, lo:],
                        start=(ki == 0), stop=(ki == nk - 1),
                        skip_group_check=(lo > 0 or ki == nk - 1),
                    )

                for ki in range(nk):
                    j = ki - 4 * qc    # >=0 on diagonal-crossing tiles
                    lo = P * j if j > 0 else 0
                    # both heads' score tiles share one 2-bank PSUM tile
                    # so a single EXP covers the pair
                    stp = st_ps.tile([P, 2, NQ], F32, tag="st",
                                     name=f"st_{qc}_{pair}_{ki}")
                    ptp = ptpool.tile([P, 2, NQ], BF16, tag="pt",
                                      name=f"pt_{qc}_{pair}_{ki}")
                    for idx, h in enumerate(heads):
                        hp = (h % 2) * 64
                        ec = h // 2
                        nc.tensor.matmul(
                            stp[:, idx, lo:],
                            kT_sb[hp:hp + 64, ec, ki * P:(ki + 1) * P],
                            qT_sb[hp:hp + 64, ec,
                                  qc * NQ + lo:(qc + 1) * NQ],
                            start=True, stop=True,
                        )
                        pts[(h, ki)] = (ptp[:, idx, :], j)
                    nc.scalar.activation(
                        out=ptp[:, :, lo:], in_=stp[:, :, lo:],
                        func=EXP, scale=scale)
                    if j >= 0:
                        # triangular mask on the diagonal 128-block
                        for idx in range(2):
                            nc.gpsimd.tensor_tensor(
                                out=ptp[:, idx, P * j:P * (j + 1)],
                                in0=ptp[:, idx, P * j:P * (j + 1)],
                                in1=mask_sb[:, :], op=MULT)
                    # P@V runs one ki behind the scores so the in-order
                    # PE stream never waits on the exp of the current ki
                    if ki >= 1:
                        for h in heads:
                            if ki == 1:
                                pvs[h] = pv_ps.tile([Dh + 1, NQ], F32,
                                                    name=f"po_{qc}_{h}",
                                                    tag="pv")
                            emit_pv(h, ki - 1)
                for h in heads:
                    emit_pv(h, nk - 1)

                for h in heads:
                    po = pvs[h]
                    # norm front half: evacuate PSUM and kick off the
                    # reciprocal chain (no PE instructions here)
                    ot_un = npool.tile([64, NQ], F32, tag="otun",
                                       name=f"otun_{qc}_{h}")
                    nc.vector.tensor_copy(out=ot_un[:], in_=po[0:64, :])
                    rr = npool.tile([P, NQ], F32R, tag="rr",
                                    name=f"rr_{qc}_{h}")
                    if qc == QC - 1:
                        # last phase: 2-instruction reciprocal on ScalarE
                        # (Ln then Exp(-x); both live in the same activation
                        # table set as the softmax Exp) — short latency, no
                        # DMA roundtrip, so the epilogue isn't gated on it
                        lnr = npool.tile([P, NQ], F32, tag="lnr",
                                         name=f"lnr_{qc}_{h}")
                        nc.scalar.activation(out=lnr[64:65, :],
                                             in_=po[64:65, :], func=LN)
                        nc.scalar.activation(out=rr[64:65, :],
                                             in_=lnr[64:65, :],
                                             func=EXP, scale=-1.0)
                    else:
                        # bounce the [1,512] sum row through DRAM to spread
                        # it over 64 partitions, so the reciprocal uses 64
                        # DVE lanes; DMAs ride the gpsimd queue
                        rsum = npool.tile([P, NQ], F32, tag="rsum",
                                          name=f"rsum_{qc}_{h}")
                        nc.vector.tensor_copy(out=rsum[64:65, :],
                                              in_=po[64:65, :])
                        dr1 = dpool.tile([NQ], F32,
                                         name=f"dr1_{qc}_{h}", tag="dr1")
                        nc.gpsimd.dma_start(
                            dr1[:].rearrange("(a b) -> a b", a=1),
                            rsum[64:65, :])
                        r64 = npool.tile([64, NQ // 64], F32, tag="r64",
                                         name=f"r64_{qc}_{h}")
                        nc.gpsimd.dma_start(
                            r64[:],
                            dr1[:].rearrange("(a b) -> a b", b=NQ // 64))
                        r64b = npool.tile([64, NQ // 64], F32, tag="r64b",
                                          name=f"r64b_{qc}_{h}")
                        nc.vector.reciprocal_approx_fast(out=r64b[:],
                                                         in_=r64[:])
                        dr2 = dpool.tile([NQ], F32,
                                         name=f"dr2_{qc}_{h}", tag="dr2")
                        nc.gpsimd.dma_start(
                            dr2[:].rearrange("(a b) -> a b", b=NQ // 64),
                            r64b[:])
                        rr_f = npool.tile([P, NQ], F32, tag="rrf",
                                          name=f"rrf_{qc}_{h}")
                        nc.gpsimd.dma_start(
                            rr_f[64:65, :],
                            dr2[:].rearrange("(a b) -> a b", a=1))
                        nc.vector.tensor_copy(out=rr[64:65, :],
                                              in_=rr_f[64:65, :])
                    norm_state[(qc, h)] = (ot_un, rr)

            def emit_norm_back_h(qc, h):
                # back half: broadcast the reciprocal row via a K=1 matmul,
                # scale on GpSimd, DMA into oT (cross-partition move for the
                # odd half-heads)
                hp = (h % 2) * 64
                ec = h // 2
                ot_un, rr = norm_state.pop((qc, h))
                ps_bc = pv_ps.tile([64, NQ], F32, tag="pv",
                                   name=f"bc_{qc}_{h}")
                nc.tensor.matmul(ps_bc[:], ones_sb[64:65, 0:64],
                                 rr[64:65, :], start=True, stop=True)
                rs_sb = wpool.tile([64, NQ], F32, tag="rs")
                nc.vector.tensor_copy(out=rs_sb[:], in_=ps_bc[:])
                tmp = wpool.tile([64, NQ], BF16, tag="tmp")
                nc.gpsimd.tensor_tensor(out=tmp[:], in0=ot_un[:],
                                        in1=rs_sb[:], op=MULT)
                nc.gpsimd.dma_start(
                    oT_sb[hp:hp + 64, ec, qc * NQ:(qc + 1) * NQ],
                    tmp[:])

            def emit_proj_lt(lt):
                # y = oT^T @ wout (partial over heads) for this l-chunk's rows
                y_sb = wpool.tile([P, 2, NQ], F32, tag="y")
                pss = []
                for do in range(D // NQ):
                    ps = mm_ps.tile([P, NQ], F32, tag="mm",
                                    name=f"y_{lt}_{do}")
                    for ec in range(EL // P):
                        nc.tensor.matmul(
                            ps[:],
                            oT_sb[:, ec, lt * P:(lt + 1) * P],
                            wout_sb[:, ec, do * NQ:(do + 1) * NQ],
                            start=(ec == 0), stop=(ec == EL // P - 1),
                        )
                    pss.append(ps)
                # evacuate the two halves on different engines in parallel
                nc.scalar.copy(out=y_sb[:, 0, :], in_=pss[0][:])
                nc.vector.tensor_copy(out=y_sb[:, 1, :], in_=pss[1][:])
                nc.sync.dma_start(
                    out.ap()[lt * P:(lt + 1) * P, :].rearrange(
                        "p (a b) -> p a b", a=2),
                    y_sb[:])

            def emit_proj_lt_ecsplit(lt, phase):
                # epilogue helper: ec=0 only needs heads 0/1 in oT, ec=1
                # needs heads 2/3 — lets projection start before the last
                # pair's norm chains finish. y lives in a 2-bank st-pool
                # tile across both calls.
                if phase == 0:
                    y_ps = st_ps.tile([P, 2, NQ], F32, tag="st",
                                      name=f"yps_{lt}")
                    _ec_state[lt] = y_ps
                    for do in range(D // NQ):
                        nc.tensor.matmul(
                            y_ps[:, do, :],
                            oT_sb[:, 0, lt * P:(lt + 1) * P],
                            wout_sb[:, 0, do * NQ:(do + 1) * NQ],
                            start=True, stop=False,
                        )
                else:
                    y_ps = _ec_state.pop(lt)
                    for do in range(D // NQ):
                        nc.tensor.matmul(
                            y_ps[:, do, :],
                            oT_sb[:, 1, lt * P:(lt + 1) * P],
                            wout_sb[:, 1, do * NQ:(do + 1) * NQ],
                            start=False, stop=True,
                        )
                    y_sb = wpool.tile([P, 2, NQ], F32, tag="y")
                    nc.scalar.copy(out=y_sb[:, 0, :], in_=y_ps[:, 0, :])
                    nc.vector.tensor_copy(out=y_sb[:, 1, :],
                                          in_=y_ps[:, 1, :])
                    nc.sync.dma_start(
                        out.ap()[lt * P:(lt + 1) * P, :].rearrange(
                            "p (a b) -> p a b", a=2),
                        y_sb[:])

            _ec_state = {}

            # phase schedule: qkv(ph) | norm-backs(ph-1) | attn(ph,0) |
            # proj(ph-1) first half | attn(ph,1) | proj(ph-1) second half.
            # The norm-backs land right after qkv so the PE has work while
            # the last qk-evacuation copies drain; projections of the
            # previous phase fill the gaps between attention pairs.
            for ph in range(QC):
                emit_qkv(ph)
                if ph >= 1:
                    for h in range(NH):
                        emit_norm_back_h(ph - 1, h)
                emit_attn_pair(ph, 0)
                if ph >= 1:
                    emit_proj_lt(4 * (ph - 1) + 0)
                    emit_proj_lt(4 * (ph - 1) + 1)
                emit_attn_pair(ph, 1)
                if ph >= 1:
                    emit_proj_lt(4 * (ph - 1) + 2)
                    emit_proj_lt(4 * (ph - 1) + 3)

            # epilogue: pair0's norm-backs first (its reciprocals completed
            # during attn(3,1)), then the ec0 half of the last projections
            # covers pair1's norm chains, then everything finishes
            ql = QC - 1
            emit_norm_back_h(ql, 0)
            emit_norm_back_h(ql, 1)
            emit_proj_lt_ecsplit(12, 0)
            emit_proj_lt_ecsplit(13, 0)
            emit_norm_back_h(ql, 2)
            emit_norm_back_h(ql, 3)
            emit_proj_lt_ecsplit(12, 1)
            emit_proj_lt_ecsplit(14, 0)
            emit_proj_lt_ecsplit(13, 1)
            emit_proj_lt_ecsplit(15, 0)
            emit_proj_lt_ecsplit(14, 1)
            emit_proj_lt_ecsplit(15, 1)

    nc.compile()
    return nc


def _host_masks():
    k = np.arange(P)[:, None]
    q = np.arange(P)[None, :]
    return (k <= q).astype(np.float32)


def _shard(x, Wq, Wk, Wv, Wout):
    import ml_dtypes
    bf16 = ml_dtypes.bfloat16
    masks = _host_masks()
    in_maps = []
    for c in range(NCORES):
        b, g = c // NH, c % NH
        hs = slice(g * NH, (g + 1) * NH)
        in_maps.append({
            "xT": np.ascontiguousarray(x[b].T).astype(bf16),
            "wq": np.ascontiguousarray(Wq[:, hs, :].reshape(D, EL)).astype(bf16),
            "wk": np.ascontiguousarray(Wk[:, hs, :].reshape(D, EL)).astype(bf16),
            "wv": np.ascontiguousarray(Wv[:, hs, :].reshape(D, EL)).astype(bf16),
            "wout": np.ascontiguousarray(Wout[hs].reshape(EL, D)).astype(bf16),
            "masks": masks.astype(bf16),
        })
    return in_maps


_NC_CACHE = None


def _get_nc():
    global _NC_CACHE
    if _NC_CACHE is None:
        _NC_CACHE = build()
    return _NC_CACHE


def run(x, Wq, Wk, Wv, Wout, trace=False):
    nc = _get_nc()
    in_maps = _shard(np.asarray(x), np.asarray(Wq), np.asarray(Wk),
                     np.asarray(Wv), np.asarray(Wout))
    res = run_bass_kernel_spmd(nc, in_maps, core_ids=list(range(NCORES)),
                               trace=trace)
    parts = [res.results[c]["out"] for c in range(NCORES)]
    full = np.stack([
        parts[0] + parts[1] + parts[2] + parts[3],
        parts[4] + parts[5] + parts[6] + parts[7],
    ]).astype(np.float32)
    return full, res


def kernel(x, Wq, Wk, Wv, Wout):
    for _ in range(3):
        full, _ = run(x, Wq, Wk, Wv, Wout, trace=False)
        if np.isfinite(full).all():
            return full
    return full


# revision 15
# speedup vs baseline: 1.1122x; 1.0919x over previous
"""Causal multi-head attention (B=2, L=2048, D=1024, H=16, Dh=64) on 8 TRN2
NeuronCores.

Sharding: data-parallel over B (2 groups of 4 cores), tensor-parallel over H
within a group (4 heads per core). Each core computes QKV projections for its
heads, full causal attention per head (flash-style, scores kept transposed so
no on-chip transposes are needed), and a partial output projection
y_c = sum_h o_h @ Wout_h. The host sums the 4 partials per batch.

v2 restructure vs the original baseline:
  - Trapezoid streaming: scores + exp + P@V only touch columns right of the
    causal diagonal (per 128-wide k-tile), instead of memset-zeroing masked
    regions and streaming full 512-wide tiles.
  - Phase schedule: norm back-halves run at the top of the next phase (before
    attention, covering the qkv->attn dependency boundary); projections are
    split around attn(pair1); the epilogue splits the last projections by
    ec-half (ec0 only needs heads 0/1) to hide the final reciprocal chains.
  - Reciprocal of the softmax denominator: ScalarE Ln + Exp(scale=-1) for the
    last phase (2 instructions, low latency, same activation table set as the
    softmax Exp), DMA-bounce + reciprocal_approx_fast for earlier phases.
  - Engine balance: output-projection PSUM evacuations split ScalarE/DVE; the
    o*(1/sum) scale-mult runs on GpSimd; norm-path DMAs ride the gpsimd queue.
"""

import numpy as np

import concourse.bass as bass
import concourse.mybir as mybir
import concourse.tile as tile
from concourse import bacc
from concourse.bass_utils import run_bass_kernel_spmd

F32 = mybir.dt.float32
F32R = mybir.dt.float32r
BF16 = mybir.dt.bfloat16
EXP = mybir.ActivationFunctionType.Exp
MULT = mybir.AluOpType.mult

B, L, D, H = 2, 2048, 1024, 16
Dh = D // H
NCORES = 8
NH = 4            # heads per core
EL = NH * Dh      # local head dims = 256
P = 128
NQ = 512          # q-chunk width (scores free dim)
QC = L // NQ      # 4 q-chunks
DC = D // P       # 8 contraction chunks for projections
LC = 4            # xT l-chunks for QKV
NL = L // LC      # 512


def build():
    nc = bacc.Bacc("TRN2", target_bir_lowering=False, debug=False,
                   num_devices=NCORES)

    xT = nc.dram_tensor("xT", [D, L], BF16, kind="ExternalInput")
    wq = nc.dram_tensor("wq", [D, EL], BF16, kind="ExternalInput")
    wk = nc.dram_tensor("wk", [D, EL], BF16, kind="ExternalInput")
    wv = nc.dram_tensor("wv", [D, EL], BF16, kind="ExternalInput")
    wout = nc.dram_tensor("wout", [EL, D], BF16, kind="ExternalInput")
    masks = nc.dram_tensor("masks", [P, P], BF16, kind="ExternalInput")
    out = nc.dram_tensor("out", [L, D], F32, kind="ExternalOutput")

    scale = 1.0 / np.sqrt(Dh)

    with tile.TileContext(nc) as tc:
        with (
            tc.tile_pool(name="const", bufs=1) as cpool,
            tc.tile_pool(name="xt", bufs=2) as xpool,
            tc.tile_pool(name="pt", bufs=6) as ptpool,
            tc.tile_pool(name="work", bufs=3) as wpool,
            tc.tile_pool(name="norm", bufs=8) as npool,
            tc.tile_pool(name="dram", bufs=8, space="DRAM") as dpool,
            tc.tile_pool(name="mm", bufs=2, space="PSUM") as mm_ps,
            tc.tile_pool(name="st", bufs=2, space="PSUM") as st_ps,
            tc.tile_pool(name="pv", bufs=2, space="PSUM") as pv_ps,
        ):
            # ---- persistent SBUF tensors ----
            wq_sb = cpool.tile([P, DC, EL], BF16, tag="wq")
            wk_sb = cpool.tile([P, DC, EL], BF16, tag="wk")
            wv_sb = cpool.tile([P, DC, EL], BF16, tag="wv")
            wout_sb = cpool.tile([P, EL // P, D], BF16, tag="wout")
            mask_sb = cpool.tile([P, P], BF16, tag="mask")
            mask2_sb = cpool.tile([P, 2, P], BF16, tag="mask2")
            qT_sb = cpool.tile([P, EL // P, L], BF16, tag="qT")
            kT_sb = cpool.tile([P, EL // P, L], BF16, tag="kT")
            vext_sb = cpool.tile([P, L // P, NH, Dh + 1], BF16, tag="vext")
            oT_sb = cpool.tile([P, EL // P, L], BF16, tag="oT")
            ones_f32 = cpool.tile([P, P], F32, tag="onesf")
            ones_sb = cpool.tile([P, P], F32R, tag="ones")

            # DMA order matters at startup: the first QKV matmul group needs
            # wq + the first xT chunk; everything else can trickle in behind
            xT_r = xT.ap().rearrange("(o p) l -> p o l", p=P)
            wq_r = wq.ap().rearrange("(o p) e -> p o e", p=P)
            xt0 = xpool.tile([P, DC, NL], BF16, tag="xt", name="xt0")
            for dc in range(0, DC, 2):
                nc.sync.dma_start(wq_sb[:, dc:dc + 2, :], wq_r[:, dc:dc + 2, :])
                nc.sync.dma_start(xt0[:, dc:dc + 2, :], xT_r[:, dc:dc + 2, 0:NL])
            nc.sync.dma_start(
                wk_sb[:], wk.ap().rearrange("(o p) e -> p o e", p=P))
            nc.sync.dma_start(
                wv_sb[:], wv.ap().rearrange("(o p) e -> p o e", p=P))
            nc.sync.dma_start(
                wout_sb[:], wout.ap().rearrange("(o p) d -> p o d", p=P))
            nc.sync.dma_start(mask_sb[:], masks[:, :])

            nc.vector.memset(ones_f32[:], 1.0)
            nc.vector.tensor_copy(out=ones_sb[:], in_=ones_f32[:])
            nc.vector.tensor_copy(out=mask2_sb[:, 0, :], in_=mask_sb[:, :])
            nc.vector.tensor_copy(out=mask2_sb[:, 1, :], in_=mask_sb[:, :])
            # ones column of vext (the softmax-denominator row of P@V)
            nc.vector.tensor_copy(
                out=vext_sb[:, :, :, Dh],
                in_=ones_f32[:, 0:L // P * NH].rearrange("p (a b) -> p a b", a=L // P),
            )

            def emit_qkv(lc):
                if lc == 0:
                    xt = xt0
                else:
                    xt = xpool.tile([P, DC, NL], BF16, tag="xt",
                                    name=f"xt{lc}")
                    for dc in range(0, DC, 2):
                        nc.sync.dma_start(
                            xt[:, dc:dc + 2, :],
                            xT_r[:, dc:dc + 2, lc * NL:(lc + 1) * NL])

                for w_sb, dst in ((wq_sb, qT_sb), (wk_sb, kT_sb)):
                    for ec in range(EL // P):
                        ps = mm_ps.tile([P, NQ], F32, tag="mm",
                                        name=f"qk_{lc}_{ec}")
                        for dc in range(DC):
                            nc.tensor.matmul(
                                ps[:],
                                w_sb[:, dc, ec * P:(ec + 1) * P],
                                xt[:, dc, :],
                                start=(dc == 0), stop=(dc == DC - 1),
                            )
                        nc.vector.tensor_copy(
                            out=dst[:, ec, lc * NL:(lc + 1) * NL], in_=ps[:])

                # previous phase's norm back-halves slot in here: their
                # reciprocal rows are ready (chains ran during the previous
                # attention pair), and they keep the PE busy while the last
                # qk evacuation copies drain
                if lc >= 1:
                    for h in range(NH):
                        emit_norm_back_h(lc - 1, h)

                for lt in range(NL // P):
                    lo = lc * (NL // P) + lt
                    ps = mm_ps.tile([P, EL], F32, tag="mm",
                                    name=f"v_{lc}_{lt}")
                    for dc in range(DC):
                        nc.tensor.matmul(
                            ps[:],
                            xt[:, dc, lt * P:(lt + 1) * P],
                            wv_sb[:, dc, :],
                            start=(dc == 0), stop=(dc == DC - 1),
                        )
                    nc.vector.tensor_copy(
                        out=vext_sb[:, lo, :, 0:Dh],
                        in_=ps[:].rearrange("p (h e) -> p h e", h=NH),
                    )

            norm_state = {}        # (qc, h) -> (ot_un, rr)

            def emit_attn_pair(qc, pair):
                nk = 4 * (qc + 1)          # causal k-chunks of 128
                heads = (2 * pair, 2 * pair + 1)
                pts = {}               # (h, ki) -> (pt AP [P, NQ], j)
                pvs = {}               # h -> accumulating PSUM tile

                def emit_pv(h, ki):
                    ap, j = pts.pop((h, ki))
                    lo = P * j if j > 0 else 0
                    nc.tensor.matmul(
                        pvs[h][:, lo:],
                        vext_sb[:, ki, h, :],
                        ap[:, lo:],
                        start=(ki == 0), stop=(ki == nk - 1),
                        skip_group_check=(lo > 0 or ki == nk - 1),
                    )

                for ki in range(nk):
                    j = ki - 4 * qc    # >=0 on diagonal-crossing tiles
                    lo = P * j if j > 0 else 0
                    # both heads' score tiles share one 2-bank PSUM tile
                    # so a single EXP covers the pair
                    stp = st_ps.tile([P, 2, NQ], F32, tag="st",
                                     name=f"st_{qc}_{pair}_{ki}")
                    ptp = ptpool.tile([P, 2, NQ], BF16, tag="pt",
                                      name=f"pt_{qc}_{pair}_{ki}")
                    for idx, h in enumerate(heads):
                        hp = (h % 2) * 64
                        ec = h // 2
                        nc.tensor.matmul(
                            stp[:, idx, lo:],
                            kT_sb[hp:hp + 64, ec, ki * P:(ki + 1) * P],
                            qT_sb[hp:hp + 64, ec,
                                  qc * NQ + lo:(qc + 1) * NQ],
                            start=True, stop=True,
                        )
                        pts[(h, ki)] = (ptp[:, idx, :], j)
                    nc.scalar.activation(
                        out=ptp[:, :, lo:], in_=stp[:, :, lo:],
                        func=EXP, scale=scale)
                    if j >= 0:
                        # triangular mask on the diagonal 128-block; bf16
                        # SBUF-to-SBUF tensor_tensor runs 2x-packed on DVE
                        for idx in range(2):
                            nc.vector.tensor_tensor(
                                out=ptp[:, idx, P * j:P * (j + 1)],
                                in0=ptp[:, idx, P * j:P * (j + 1)],
                                in1=mask_sb[:, :],
                                op=MULT)
                    # P@V runs one ki behind the scores so the in-order
                    # PE stream never waits on the exp of the current ki
                    if ki >= 1:
                        for h in heads:
                            if ki == 1:
                                pvs[h] = pv_ps.tile([Dh + 1, NQ], F32,
                                                    name=f"po_{qc}_{h}",
                                                    tag="pv")
                            emit_pv(h, ki - 1)
                for h in heads:
                    emit_pv(h, nk - 1)

                # norm front half, batched over the pair so the DVE never
                # parks behind a DMA roundtrip: evacuate both heads' PSUM,
                # then run both reciprocal chains
                fr = {}
                for h in heads:
                    po = pvs[h]
                    ot_un = npool.tile([64, NQ], F32, tag="otun",
                                       name=f"otun_{qc}_{h}")
                    nc.vector.tensor_copy(out=ot_un[:], in_=po[0:64, :])
                    rr = npool.tile([P, NQ], F32R, tag="rr",
                                    name=f"rr_{qc}_{h}")
                    rsum = npool.tile([P, NQ], F32, tag="rsum",
                                      name=f"rsum_{qc}_{h}")
                    nc.vector.tensor_copy(out=rsum[64:65, :],
                                          in_=po[64:65, :])
                    fr[h] = (ot_un, rr, rsum)
                    norm_state[(qc, h)] = (ot_un, rr)
                if qc == QC - 1:
                    # last phase: direct reciprocal on the [1,512] row —
                    # slow per-element but no DMA roundtrip, so the
                    # epilogue isn't gated on it
                    for h in heads:
                        ot_un, rr, rsum = fr[h]
                        rr_f = npool.tile([P, NQ], F32, tag="rrf",
                                          name=f"rrf_{qc}_{h}")
                        nc.vector.reciprocal(rr_f[64:65, :],
                                             rsum[64:65, :])
                        nc.vector.tensor_copy(out=rr[64:65, :],
                                              in_=rr_f[64:65, :])
                else:
                    # bounce the [1,512] sum row through DRAM to spread it
                    # over 64 partitions, so the reciprocal uses 64 DVE
                    # lanes; both heads' DMAs issue before either reciprocal
                    r64s = {}
                    for h in heads:
                        _, _, rsum = fr[h]
                        dr1 = dpool.tile([NQ], F32,
                                         name=f"dr1_{qc}_{h}", tag="dr1")
                        nc.sync.dma_start(
                            dr1[:].rearrange("(a b) -> a b", a=1),
                            rsum[64:65, :])
                        r64 = npool.tile([64, NQ // 64], F32, tag="r64",
                                         name=f"r64_{qc}_{h}")
                        nc.sync.dma_start(
                            r64[:],
                            dr1[:].rearrange("(a b) -> a b", b=NQ // 64))
                        r64s[h] = r64
                    for h in heads:
                        r64b = npool.tile([64, NQ // 64], F32, tag="r64b",
                                          name=f"r64b_{qc}_{h}")
                        nc.vector.reciprocal_approx_fast(out=r64b[:],
                                                         in_=r64s[h][:])
                        r64s[h] = r64b
                    rrfs = {}
                    for h in heads:
                        dr2 = dpool.tile([NQ], F32,
                                         name=f"dr2_{qc}_{h}", tag="dr2")
                        nc.sync.dma_start(
                            dr2[:].rearrange("(a b) -> a b", b=NQ // 64),
                            r64s[h][:])
                        rr_f = npool.tile([P, NQ], F32, tag="rrf",
                                          name=f"rrf_{qc}_{h}")
                        nc.sync.dma_start(
                            rr_f[64:65, :],
                            dr2[:].rearrange("(a b) -> a b", a=1))
                        rrfs[h] = rr_f
                    for h in heads:
                        _, rr, _ = fr[h]
                        nc.vector.tensor_copy(out=rr[64:65, :],
                                              in_=rrfs[h][64:65, :])

            def emit_norm_back_h(qc, h):
                # back half: broadcast the reciprocal row via a K=1 matmul,
                # scale on GpSimd, DMA into oT (cross-partition move for the
                # odd half-heads)
                hp = (h % 2) * 64
                ec = h // 2
                ot_un, rr = norm_state.pop((qc, h))
                ps_bc = pv_ps.tile([64, NQ], F32, tag="pv",
                                   name=f"bc_{qc}_{h}")
                nc.tensor.matmul(ps_bc[:], ones_sb[64:65, 0:64],
                                 rr[64:65, :], start=True, stop=True)
                rs_sb = wpool.tile([64, NQ], F32, tag="rs")
                nc.vector.tensor_copy(out=rs_sb[:], in_=ps_bc[:])
                tmp = wpool.tile([64, NQ], BF16, tag="tmp")
                nc.gpsimd.tensor_tensor(out=tmp[:], in0=ot_un[:],
                                        in1=rs_sb[:], op=MULT)
                nc.gpsimd.dma_start(
                    oT_sb[hp:hp + 64, ec, qc * NQ:(qc + 1) * NQ],
                    tmp[:])

            def emit_proj_lt(lt):
                # y = oT^T @ wout (partial over heads) for this l-chunk's rows
                y_sb = wpool.tile([P, 2, NQ], F32, tag="y")
                pss = []
                for do in range(D // NQ):
                    ps = mm_ps.tile([P, NQ], F32, tag="mm",
                                    name=f"y_{lt}_{do}")
                    for ec in range(EL // P):
                        nc.tensor.matmul(
                            ps[:],
                            oT_sb[:, ec, lt * P:(lt + 1) * P],
                            wout_sb[:, ec, do * NQ:(do + 1) * NQ],
                            start=(ec == 0), stop=(ec == EL // P - 1),
                        )
                    pss.append(ps)
                # evacuate the two halves on different engines in parallel
                nc.scalar.copy(out=y_sb[:, 0, :], in_=pss[0][:])
                nc.vector.tensor_copy(out=y_sb[:, 1, :], in_=pss[1][:])
                nc.sync.dma_start(
                    out.ap()[lt * P:(lt + 1) * P, :].rearrange(
                        "p (a b) -> p a b", a=2),
                    y_sb[:])

            def emit_proj_lt_ecsplit(lt, phase):
                # epilogue helper: ec=0 only needs heads 0/1 in oT, ec=1
                # needs heads 2/3 — lets projection start before the last
                # pair's norm chains finish. The four concurrent
                # accumulators are spread over the st/mm/pv pools (8 banks).
                if phase == 0:
                    if lt < 14:
                        yp = st_ps.tile([P, 2, NQ], F32, tag="st",
                                        name=f"yps_{lt}")
                        pss = [yp[:, 0, :], yp[:, 1, :]]
                    else:
                        pss = [mm_ps.tile([P, NQ], F32, tag="mm",
                                          name=f"yps_{lt}_{do}")[:]
                               for do in range(2)]
                    _ec_state[lt] = pss
                    for do in range(D // NQ):
                        nc.tensor.matmul(
                            pss[do],
                            oT_sb[:, 0, lt * P:(lt + 1) * P],
                            wout_sb[:, 0, do * NQ:(do + 1) * NQ],
                            start=True, stop=False,
                        )
                else:
                    pss = _ec_state.pop(lt)
                    for do in range(D // NQ):
                        nc.tensor.matmul(
                            pss[do],
                            oT_sb[:, 1, lt * P:(lt + 1) * P],
                            wout_sb[:, 1, do * NQ:(do + 1) * NQ],
                            start=False, stop=True,
                        )
                    y_sb = wpool.tile([P, 2, NQ], F32, tag="y")
                    nc.scalar.copy(out=y_sb[:, 0, :], in_=pss[0])
                    nc.vector.tensor_copy(out=y_sb[:, 1, :], in_=pss[1])
                    nc.sync.dma_start(
                        out.ap()[lt * P:(lt + 1) * P, :].rearrange(
                            "p (a b) -> p a b", a=2),
                        y_sb[:])

            _ec_state = {}

            # phase schedule: qkv(ph) (norm-backs of ph-1 interleaved after
            # the qk chains) | attn(ph,0) | proj(ph-1) first half |
            # attn(ph,1) | proj(ph-1) second half. Projections of the
            # previous phase fill the gaps between attention pairs.
            for ph in range(QC):
                emit_qkv(ph)
                emit_attn_pair(ph, 0)
                if ph >= 1:
                    emit_proj_lt(4 * (ph - 1) + 0)
                    emit_proj_lt(4 * (ph - 1) + 1)
                emit_attn_pair(ph, 1)
                if ph >= 1:
                    emit_proj_lt(4 * (ph - 1) + 2)
                    emit_proj_lt(4 * (ph - 1) + 3)

            # epilogue: pair0's norm-backs first (its reciprocals completed
            # during attn(3,1)), then the ec0 halves of the next three
            # projections (they only need heads 0/1) cover pair1's norm
            # chains, then everything finishes; lt15 reuses the mm pool
            # after lt14 frees it
            ql = QC - 1
            emit_norm_back_h(ql, 0)
            emit_norm_back_h(ql, 1)
            for lt in range(12, 15):
                emit_proj_lt_ecsplit(lt, 0)
            emit_norm_back_h(ql, 2)
            emit_norm_back_h(ql, 3)
            for lt in range(12, 15):
                emit_proj_lt_ecsplit(lt, 1)
            emit_proj_lt(15)

    nc.compile()
    return nc


def _host_masks():
    k = np.arange(P)[:, None]
    q = np.arange(P)[None, :]
    return (k <= q).astype(np.float32)


def _shard(x, Wq, Wk, Wv, Wout):
    import ml_dtypes
    bf16 = ml_dtypes.bfloat16
    masks = _host_masks()
    in_maps = []
    for c in range(NCORES):
        b, g = c // NH, c % NH
        hs = slice(g * NH, (g + 1) * NH)
        in_maps.append({
            "xT": np.ascontiguousarray(x[b].T).astype(bf16),
            "wq": np.ascontiguousarray(Wq[:, hs, :].reshape(D, EL)).astype(bf16),
            "wk": np.ascontiguousarray(Wk[:, hs, :].reshape(D, EL)).astype(bf16),
            "wv": np.ascontiguousarray(Wv[:, hs, :].reshape(D, EL)).astype(bf16),
            "wout": np.ascontiguousarray(Wout[hs].reshape(EL, D)).astype(bf16),
            "masks": masks.astype(bf16),
        })
    return in_maps


_NC_CACHE = None


def _get_nc():
    global _NC_CACHE
    if _NC_CACHE is None:
        _NC_CACHE = build()
    return _NC_CACHE


def run(x, Wq, Wk, Wv, Wout, trace=False):
    nc = _get_nc()
    in_maps = _shard(np.asarray(x), np.asarray(Wq), np.asarray(Wk),
                     np.asarray(Wv), np.asarray(Wout))
    res = run_bass_kernel_spmd(nc, in_maps, core_ids=list(range(NCORES)),
                               trace=trace)
    parts = [res.results[c]["out"] for c in range(NCORES)]
    full = np.stack([
        parts[0] + parts[1] + parts[2] + parts[3],
        parts[4] + parts[5] + parts[6] + parts[7],
    ]).astype(np.float32)
    return full, res


def kernel(x, Wq, Wk, Wv, Wout):
    for _ in range(3):
        full, _ = run(x, Wq, Wk, Wv, Wout, trace=False)
        if np.isfinite(full).all():
            return full
    return full


# revision 22
# speedup vs baseline: 1.1180x; 1.0053x over previous
"""Causal multi-head attention (B=2, L=2048, D=1024, H=16, Dh=64) on 8 TRN2
NeuronCores.

Sharding: data-parallel over B (2 groups of 4 cores), tensor-parallel over H
within a group (4 heads per core). Each core computes QKV projections for its
heads, full causal attention per head (flash-style, scores kept transposed so
no on-chip transposes are needed), and a partial output projection
y_c = sum_h o_h @ Wout_h. The host sums the 4 partials per batch.

v2 restructure vs the original baseline:
  - Trapezoid streaming: scores + exp + P@V only touch columns right of the
    causal diagonal (per 128-wide k-tile), instead of memset-zeroing masked
    regions and streaming full 512-wide tiles.
  - Phase schedule: norm back-halves run at the top of the next phase (before
    attention, covering the qkv->attn dependency boundary); projections are
    split around attn(pair1); the epilogue splits the last projections by
    ec-half (ec0 only needs heads 0/1) to hide the final reciprocal chains.
  - Reciprocal of the softmax denominator: ScalarE Ln + Exp(scale=-1) for the
    last phase (2 instructions, low latency, same activation table set as the
    softmax Exp), DMA-bounce + reciprocal_approx_fast for earlier phases.
  - Engine balance: output-projection PSUM evacuations split ScalarE/DVE; the
    o*(1/sum) scale-mult runs on GpSimd; norm-path DMAs ride the gpsimd queue.
"""

import numpy as np

import concourse.bass as bass
import concourse.mybir as mybir
import concourse.tile as tile
from concourse import bacc
from concourse.bass_utils import run_bass_kernel_spmd

F32 = mybir.dt.float32
F32R = mybir.dt.float32r
BF16 = mybir.dt.bfloat16
EXP = mybir.ActivationFunctionType.Exp
MULT = mybir.AluOpType.mult

B, L, D, H = 2, 2048, 1024, 16
Dh = D // H
NCORES = 8
NH = 4            # heads per core
EL = NH * Dh      # local head dims = 256
P = 128
NQ = 512          # q-chunk width (scores free dim)
QC = L // NQ      # 4 q-chunks
DC = D // P       # 8 contraction chunks for projections
LC = 4            # xT l-chunks for QKV
NL = L // LC      # 512


def build():
    nc = bacc.Bacc("TRN2", target_bir_lowering=False, debug=False,
                   num_devices=NCORES)

    xT = nc.dram_tensor("xT", [D, L], BF16, kind="ExternalInput")
    wq = nc.dram_tensor("wq", [D, EL], BF16, kind="ExternalInput")
    wk = nc.dram_tensor("wk", [D, EL], BF16, kind="ExternalInput")
    wv = nc.dram_tensor("wv", [D, EL], BF16, kind="ExternalInput")
    wout = nc.dram_tensor("wout", [EL, D], BF16, kind="ExternalInput")
    masks = nc.dram_tensor("masks", [P, P], BF16, kind="ExternalInput")
    out = nc.dram_tensor("out", [L, D], F32, kind="ExternalOutput")

    scale = 1.0 / np.sqrt(Dh)

    with tile.TileContext(nc) as tc:
        with (
            tc.tile_pool(name="const", bufs=1) as cpool,
            tc.tile_pool(name="xt", bufs=2) as xpool,
            tc.tile_pool(name="pt", bufs=6) as ptpool,
            tc.tile_pool(name="work", bufs=3) as wpool,
            tc.tile_pool(name="norm", bufs=8) as npool,
            tc.tile_pool(name="dram", bufs=8, space="DRAM") as dpool,
            tc.tile_pool(name="mm", bufs=2, space="PSUM") as mm_ps,
            tc.tile_pool(name="st", bufs=2, space="PSUM") as st_ps,
            tc.tile_pool(name="pv", bufs=2, space="PSUM") as pv_ps,
        ):
            # ---- persistent SBUF tensors ----
            wq_sb = cpool.tile([P, DC, EL], BF16, tag="wq")
            wk_sb = cpool.tile([P, DC, EL], BF16, tag="wk")
            wv_sb = cpool.tile([P, DC, EL], BF16, tag="wv")
            wout_sb = cpool.tile([P, EL // P, D], BF16, tag="wout")
            mask_sb = cpool.tile([P, P], BF16, tag="mask")
            mask2_sb = cpool.tile([P, 2, P], BF16, tag="mask2")
            qT_sb = cpool.tile([P, EL // P, L], BF16, tag="qT")
            kT_sb = cpool.tile([P, EL // P, L], BF16, tag="kT")
            vext_sb = cpool.tile([P, L // P, NH, Dh + 1], BF16, tag="vext")
            oT_sb = cpool.tile([P, EL // P, L], BF16, tag="oT")
            ones_f32 = cpool.tile([P, P], F32, tag="onesf")
            ones_sb = cpool.tile([P, P], F32R, tag="ones")

            # DMA order matters at startup: the first QKV matmul group needs
            # wq + the first xT chunk; everything else can trickle in behind
            xT_r = xT.ap().rearrange("(o p) l -> p o l", p=P)
            wq_r = wq.ap().rearrange("(o p) e -> p o e", p=P)
            xt0 = xpool.tile([P, DC, NL], BF16, tag="xt", name="xt0")
            for dc in range(0, DC, 2):
                nc.sync.dma_start(wq_sb[:, dc:dc + 2, :], wq_r[:, dc:dc + 2, :])
                nc.sync.dma_start(xt0[:, dc:dc + 2, :], xT_r[:, dc:dc + 2, 0:NL])
            nc.sync.dma_start(
                wk_sb[:], wk.ap().rearrange("(o p) e -> p o e", p=P))
            nc.sync.dma_start(
                wv_sb[:], wv.ap().rearrange("(o p) e -> p o e", p=P))
            nc.sync.dma_start(
                wout_sb[:], wout.ap().rearrange("(o p) d -> p o d", p=P))
            nc.sync.dma_start(mask_sb[:], masks[:, :])

            nc.vector.memset(ones_f32[:], 1.0)
            nc.vector.tensor_copy(out=ones_sb[:], in_=ones_f32[:])
            nc.vector.tensor_copy(out=mask2_sb[:, 0, :], in_=mask_sb[:, :])
            nc.vector.tensor_copy(out=mask2_sb[:, 1, :], in_=mask_sb[:, :])
            # ones column of vext (the softmax-denominator row of P@V)
            nc.vector.tensor_copy(
                out=vext_sb[:, :, :, Dh],
                in_=ones_f32[:, 0:L // P * NH].rearrange("p (a b) -> p a b", a=L // P),
            )

            def emit_qkv(lc):
                if lc == 0:
                    xt = xt0
                else:
                    xt = xpool.tile([P, DC, NL], BF16, tag="xt",
                                    name=f"xt{lc}")
                    for dc in range(0, DC, 2):
                        nc.sync.dma_start(
                            xt[:, dc:dc + 2, :],
                            xT_r[:, dc:dc + 2, lc * NL:(lc + 1) * NL])

                for w_sb, dst in ((wq_sb, qT_sb), (wk_sb, kT_sb)):
                    for ec in range(EL // P):
                        ps = mm_ps.tile([P, NQ], F32, tag="mm",
                                        name=f"qk_{lc}_{ec}")
                        for dc in range(DC):
                            nc.tensor.matmul(
                                ps[:],
                                w_sb[:, dc, ec * P:(ec + 1) * P],
                                xt[:, dc, :],
                                start=(dc == 0), stop=(dc == DC - 1),
                            )
                        nc.vector.tensor_copy(
                            out=dst[:, ec, lc * NL:(lc + 1) * NL], in_=ps[:])

                for lt in range(NL // P):
                    lo = lc * (NL // P) + lt
                    ps = mm_ps.tile([P, EL], F32, tag="mm",
                                    name=f"v_{lc}_{lt}")
                    for dc in range(DC):
                        nc.tensor.matmul(
                            ps[:],
                            xt[:, dc, lt * P:(lt + 1) * P],
                            wv_sb[:, dc, :],
                            start=(dc == 0), stop=(dc == DC - 1),
                        )
                    nc.vector.tensor_copy(
                        out=vext_sb[:, lo, :, 0:Dh],
                        in_=ps[:].rearrange("p (h e) -> p h e", h=NH),
                    )

                # previous phase's norm back-halves go after the v chains:
                # the v chains cover the qk-evacuation boundary, and the
                # extra time lets the reciprocal DMA bounces finish
                if lc >= 1:
                    for h in range(NH):
                        emit_norm_back_h(lc - 1, h)

            norm_state = {}        # (qc, h) -> (ot_un, rr)

            def emit_attn_pair(qc, pair):
                nk = 4 * (qc + 1)          # causal k-chunks of 128
                heads = (2 * pair, 2 * pair + 1)
                pts = {}               # (h, ki) -> (pt AP [P, NQ], j)
                pvs = {}               # h -> accumulating PSUM tile

                def emit_pv(h, ki):
                    ap, j = pts.pop((h, ki))
                    lo = P * j if j > 0 else 0
                    nc.tensor.matmul(
                        pvs[h][:, lo:],
                        vext_sb[:, ki, h, :],
                        ap[:, lo:],
                        start=(ki == 0), stop=(ki == nk - 1),
                        skip_group_check=(lo > 0 or ki == nk - 1),
                    )

                for ki in range(nk):
                    j = ki - 4 * qc    # >=0 on diagonal-crossing tiles
                    lo = P * j if j > 0 else 0
                    # both heads' score tiles share one 2-bank PSUM tile
                    # so a single EXP covers the pair
                    stp = st_ps.tile([P, 2, NQ], F32, tag="st",
                                     name=f"st_{qc}_{pair}_{ki}")
                    ptp = ptpool.tile([P, 2, NQ], BF16, tag="pt",
                                      name=f"pt_{qc}_{pair}_{ki}")
                    for idx, h in enumerate(heads):
                        hp = (h % 2) * 64
                        ec = h // 2
                        nc.tensor.matmul(
                            stp[:, idx, lo:],
                            kT_sb[hp:hp + 64, ec, ki * P:(ki + 1) * P],
                            qT_sb[hp:hp + 64, ec,
                                  qc * NQ + lo:(qc + 1) * NQ],
                            start=True, stop=True,
                        )
                        pts[(h, ki)] = (ptp[:, idx, :], j)
                    nc.scalar.activation(
                        out=ptp[:, :, lo:], in_=stp[:, :, lo:],
                        func=EXP, scale=scale)
                    if j >= 0:
                        # triangular mask on the diagonal 128-block; bf16
                        # SBUF-to-SBUF tensor_tensor runs 2x-packed on DVE
                        for idx in range(2):
                            nc.vector.tensor_tensor(
                                out=ptp[:, idx, P * j:P * (j + 1)],
                                in0=ptp[:, idx, P * j:P * (j + 1)],
                                in1=mask_sb[:, :],
                                op=MULT)
                    # P@V runs one ki behind the scores so the in-order
                    # PE stream never waits on the exp of the current ki
                    if ki >= 1:
                        for h in heads:
                            if ki == 1:
                                pvs[h] = pv_ps.tile([Dh + 1, NQ], F32,
                                                    name=f"po_{qc}_{h}",
                                                    tag="pv")
                            emit_pv(h, ki - 1)
                for h in heads:
                    emit_pv(h, nk - 1)

                # norm front half, batched over the pair so the DVE never
                # parks behind a DMA roundtrip: evacuate both heads' PSUM,
                # then run both reciprocal chains
                fr = {}
                for h in heads:
                    po = pvs[h]
                    ot_un = npool.tile([64, NQ], F32, tag="otun",
                                       name=f"otun_{qc}_{h}")
                    nc.vector.tensor_copy(out=ot_un[:], in_=po[0:64, :])
                    rr = npool.tile([P, NQ], F32R, tag="rr",
                                    name=f"rr_{qc}_{h}")
                    rsum = npool.tile([P, NQ], F32, tag="rsum",
                                      name=f"rsum_{qc}_{h}")
                    nc.vector.tensor_copy(out=rsum[64:65, :],
                                          in_=po[64:65, :])
                    fr[h] = (ot_un, rr, rsum)
                    norm_state[(qc, h)] = (ot_un, rr)
                if True:
                    # bounce the [1,512] sum row through DRAM to spread it
                    # over 64 partitions, so the reciprocal uses 64 DVE
                    # lanes; both heads' DMAs issue before either reciprocal
                    r64s = {}
                    for h in heads:
                        _, _, rsum = fr[h]
                        dr1 = dpool.tile([NQ], F32,
                                         name=f"dr1_{qc}_{h}", tag="dr1")
                        nc.sync.dma_start(
                            dr1[:].rearrange("(a b) -> a b", a=1),
                            rsum[64:65, :])
                        r64 = npool.tile([64, NQ // 64], F32, tag="r64",
                                         name=f"r64_{qc}_{h}")
                        nc.sync.dma_start(
                            r64[:],
                            dr1[:].rearrange("(a b) -> a b", b=NQ // 64))
                        r64s[h] = r64
                    for h in heads:
                        r64b = npool.tile([64, NQ // 64], F32, tag="r64b",
                                          name=f"r64b_{qc}_{h}")
                        nc.vector.reciprocal_approx_fast(out=r64b[:],
                                                         in_=r64s[h][:])
                        r64s[h] = r64b
                    rrfs = {}
                    for h in heads:
                        dr2 = dpool.tile([NQ], F32,
                                         name=f"dr2_{qc}_{h}", tag="dr2")
                        nc.sync.dma_start(
                            dr2[:].rearrange("(a b) -> a b", b=NQ // 64),
                            r64s[h][:])
                        rr_f = npool.tile([P, NQ], F32, tag="rrf",
                                          name=f"rrf_{qc}_{h}")
                        nc.sync.dma_start(
                            rr_f[64:65, :],
                            dr2[:].rearrange("(a b) -> a b", a=1))
                        rrfs[h] = rr_f
                    for h in heads:
                        _, rr, _ = fr[h]
                        nc.vector.tensor_copy(out=rr[64:65, :],
                                              in_=rrfs[h][64:65, :])

            def emit_norm_back_h(qc, h, fast=False):
                # back half: broadcast the reciprocal row via a K=1 matmul,
                # scale, DMA into oT (cross-partition move for the odd
                # half-heads). fast=True (epilogue) keeps the whole chain on
                # DVE + sync for the shortest latency; the in-phase variant
                # offloads the scale-mult to GpSimd.
                hp = (h % 2) * 64
                ec = h // 2
                ot_un, rr = norm_state.pop((qc, h))
                ps_bc = pv_ps.tile([64, NQ], F32, tag="pv",
                                   name=f"bc_{qc}_{h}")
                nc.tensor.matmul(ps_bc[:], ones_sb[64:65, 0:64],
                                 rr[64:65, :], start=True, stop=True)
                rs_sb = wpool.tile([64, NQ], F32, tag="rs")
                nc.vector.tensor_copy(out=rs_sb[:], in_=ps_bc[:])
                tmp = wpool.tile([64, NQ], BF16, tag="tmp")
                if fast:
                    nc.vector.tensor_tensor(out=tmp[:], in0=ot_un[:],
                                            in1=rs_sb[:], op=MULT)
                    nc.sync.dma_start(
                        oT_sb[hp:hp + 64, ec, qc * NQ:(qc + 1) * NQ],
                        tmp[:])
                else:
                    nc.gpsimd.tensor_tensor(out=tmp[:], in0=ot_un[:],
                                            in1=rs_sb[:], op=MULT)
                    nc.gpsimd.dma_start(
                        oT_sb[hp:hp + 64, ec, qc * NQ:(qc + 1) * NQ],
                        tmp[:])

            def emit_proj_lt(lt):
                # y = oT^T @ wout (partial over heads) for this l-chunk's rows
                y_sb = wpool.tile([P, 2, NQ], F32, tag="y")
                pss = []
                for do in range(D // NQ):
                    ps = mm_ps.tile([P, NQ], F32, tag="mm",
                                    name=f"y_{lt}_{do}")
                    for ec in range(EL // P):
                        nc.tensor.matmul(
                            ps[:],
                            oT_sb[:, ec, lt * P:(lt + 1) * P],
                            wout_sb[:, ec, do * NQ:(do + 1) * NQ],
                            start=(ec == 0), stop=(ec == EL // P - 1),
                        )
                    pss.append(ps)
                # evacuate the two halves on different engines in parallel
                nc.scalar.copy(out=y_sb[:, 0, :], in_=pss[0][:])
                nc.vector.tensor_copy(out=y_sb[:, 1, :], in_=pss[1][:])
                nc.sync.dma_start(
                    out.ap()[lt * P:(lt + 1) * P, :].rearrange(
                        "p (a b) -> p a b", a=2),
                    y_sb[:])

            def emit_proj_lt_ecsplit(lt, phase):
                # epilogue helper: ec=0 only needs heads 0/1 in oT, ec=1
                # needs heads 2/3 — lets projection start before the last
                # pair's norm chains finish. The four concurrent
                # accumulators are spread over the st/mm/pv pools (8 banks).
                if phase == 0:
                    yp = st_ps.tile([P, 2, NQ], F32, tag="st",
                                    name=f"yps_{lt}")
                    pss = [yp[:, 0, :], yp[:, 1, :]]
                    _ec_state[lt] = pss
                    for do in range(D // NQ):
                        nc.tensor.matmul(
                            pss[do],
                            oT_sb[:, 0, lt * P:(lt + 1) * P],
                            wout_sb[:, 0, do * NQ:(do + 1) * NQ],
                            start=True, stop=False,
                        )
                else:
                    pss = _ec_state.pop(lt)
                    for do in range(D // NQ):
                        nc.tensor.matmul(
                            pss[do],
                            oT_sb[:, 1, lt * P:(lt + 1) * P],
                            wout_sb[:, 1, do * NQ:(do + 1) * NQ],
                            start=False, stop=True,
                        )
                    y_sb = wpool.tile([P, 2, NQ], F32, tag="y")
                    nc.scalar.copy(out=y_sb[:, 0, :], in_=pss[0])
                    nc.vector.tensor_copy(out=y_sb[:, 1, :], in_=pss[1])
                    nc.sync.dma_start(
                        out.ap()[lt * P:(lt + 1) * P, :].rearrange(
                            "p (a b) -> p a b", a=2),
                        y_sb[:])

            _ec_state = {}

            # phase schedule: qkv(ph) (norm-backs of ph-1 interleaved after
            # the qk chains) | attn(ph,0) | proj(ph-1) first half |
            # attn(ph,1) | proj(ph-1) second half. Projections of the
            # previous phase fill the gaps between attention pairs.
            for ph in range(QC):
                emit_qkv(ph)
                emit_attn_pair(ph, 0)
                if ph >= 1:
                    emit_proj_lt(4 * (ph - 1) + 0)
                    emit_proj_lt(4 * (ph - 1) + 1)
                emit_attn_pair(ph, 1)
                if ph >= 1 and ph < QC - 1:
                    emit_proj_lt(4 * (ph - 1) + 2)
                    emit_proj_lt(4 * (ph - 1) + 3)

            # epilogue: pair0's norm-backs first (its reciprocal bounces
            # completed during attn(3,1)), then the ec0 halves of the next
            # three projections (they only need heads 0/1) plus the held-
            # back proj(2) rows cover pair1's norm chains
            ql = QC - 1
            emit_norm_back_h(ql, 0, fast=True)
            emit_norm_back_h(ql, 1, fast=True)
            emit_proj_lt_ecsplit(12, 0)
            emit_proj_lt_ecsplit(13, 0)
            emit_norm_back_h(ql, 2, fast=True)
            emit_norm_back_h(ql, 3, fast=True)
            emit_proj_lt(10)
            emit_proj_lt(11)
            emit_proj_lt_ecsplit(12, 1)
            emit_proj_lt_ecsplit(13, 1)
            emit_proj_lt(14)
            emit_proj_lt(15)

    nc.compile()
    return nc


def _host_masks():
    k = np.arange(P)[:, None]
    q = np.arange(P)[None, :]
    return (k <= q).astype(np.float32)


def _shard(x, Wq, Wk, Wv, Wout):
    import ml_dtypes
    bf16 = ml_dtypes.bfloat16
    masks = _host_masks()
    in_maps = []
    for c in range(NCORES):
        b, g = c // NH, c % NH
        hs = slice(g * NH, (g + 1) * NH)
        in_maps.append({
            "xT": np.ascontiguousarray(x[b].T).astype(bf16),
            "wq": np.ascontiguousarray(Wq[:, hs, :].reshape(D, EL)).astype(bf16),
            "wk": np.ascontiguousarray(Wk[:, hs, :].reshape(D, EL)).astype(bf16),
            "wv": np.ascontiguousarray(Wv[:, hs, :].reshape(D, EL)).astype(bf16),
            "wout": np.ascontiguousarray(Wout[hs].reshape(EL, D)).astype(bf16),
            "masks": masks.astype(bf16),
        })
    return in_maps


_NC_CACHE = None


def _get_nc():
    global _NC_CACHE
    if _NC_CACHE is None:
        _NC_CACHE = build()
    return _NC_CACHE


def run(x, Wq, Wk, Wv, Wout, trace=False):
    nc = _get_nc()
    in_maps = _shard(np.asarray(x), np.asarray(Wq), np.asarray(Wk),
                     np.asarray(Wv), np.asarray(Wout))
    res = run_bass_kernel_spmd(nc, in_maps, core_ids=list(range(NCORES)),
                               trace=trace)
    parts = [res.results[c]["out"] for c in range(NCORES)]
    full = np.stack([
        parts[0] + parts[1] + parts[2] + parts[3],
        parts[4] + parts[5] + parts[6] + parts[7],
    ]).astype(np.float32)
    return full, res


def kernel(x, Wq, Wk, Wv, Wout):
    for _ in range(3):
        full, _ = run(x, Wq, Wk, Wv, Wout, trace=False)
        if np.isfinite(full).all():
            return full
    return full


# revision 29
# speedup vs baseline: 1.1281x; 1.0090x over previous
"""Causal multi-head attention (B=2, L=2048, D=1024, H=16, Dh=64) on 8 TRN2
NeuronCores.

Sharding: data-parallel over B (2 groups of 4 cores), tensor-parallel over H
within a group (4 heads per core). Each core computes QKV projections for its
heads, full causal attention per head (flash-style, scores kept transposed so
no on-chip transposes are needed), and a partial output projection
y_c = sum_h o_h @ Wout_h. The host sums the 4 partials per batch.

v2 restructure vs the original baseline:
  - Trapezoid streaming: scores + exp + P@V only touch columns right of the
    causal diagonal (per 128-wide k-tile), instead of memset-zeroing masked
    regions and streaming full 512-wide tiles.
  - Phase schedule: norm back-halves run at the top of the next phase (before
    attention, covering the qkv->attn dependency boundary); projections are
    split around attn(pair1); the epilogue splits the last projections by
    ec-half (ec0 only needs heads 0/1) to hide the final reciprocal chains.
  - Reciprocal of the softmax denominator: ScalarE Ln + Exp(scale=-1) for the
    last phase (2 instructions, low latency, same activation table set as the
    softmax Exp), DMA-bounce + reciprocal_approx_fast for earlier phases.
  - Engine balance: output-projection PSUM evacuations split ScalarE/DVE; the
    o*(1/sum) scale-mult runs on GpSimd; norm-path DMAs ride the gpsimd queue.
"""

import numpy as np

import concourse.bass as bass
import concourse.mybir as mybir
import concourse.tile as tile
from concourse import bacc
from concourse.bass_utils import run_bass_kernel_spmd

F32 = mybir.dt.float32
F32R = mybir.dt.float32r
BF16 = mybir.dt.bfloat16
EXP = mybir.ActivationFunctionType.Exp
MULT = mybir.AluOpType.mult

B, L, D, H = 2, 2048, 1024, 16
Dh = D // H
NCORES = 8
NH = 4            # heads per core
EL = NH * Dh      # local head dims = 256
P = 128
NQ = 512          # q-chunk width (scores free dim)
QC = L // NQ      # 4 q-chunks
DC = D // P       # 8 contraction chunks for projections
LC = 4            # xT l-chunks for QKV
NL = L // LC      # 512


def build():
    nc = bacc.Bacc("TRN2", target_bir_lowering=False, debug=False,
                   num_devices=NCORES)

    xT = nc.dram_tensor("xT", [D, L], BF16, kind="ExternalInput")
    wq = nc.dram_tensor("wq", [D, EL], BF16, kind="ExternalInput")
    wk = nc.dram_tensor("wk", [D, EL], BF16, kind="ExternalInput")
    wv = nc.dram_tensor("wv", [D, EL], BF16, kind="ExternalInput")
    wout = nc.dram_tensor("wout", [EL, D], BF16, kind="ExternalInput")
    masks = nc.dram_tensor("masks", [P, P], BF16, kind="ExternalInput")
    out = nc.dram_tensor("out", [L, D], F32, kind="ExternalOutput")

    scale = 1.0 / np.sqrt(Dh)

    with tile.TileContext(nc) as tc:
        with (
            tc.tile_pool(name="const", bufs=1) as cpool,
            tc.tile_pool(name="xt", bufs=2) as xpool,
            tc.tile_pool(name="pt", bufs=6) as ptpool,
            tc.tile_pool(name="work", bufs=3) as wpool,
            tc.tile_pool(name="norm", bufs=8) as npool,
            tc.tile_pool(name="dram", bufs=8, space="DRAM") as dpool,
            tc.tile_pool(name="mm", bufs=2, space="PSUM") as mm_ps,
            tc.tile_pool(name="st", bufs=2, space="PSUM") as st_ps,
            tc.tile_pool(name="pv", bufs=2, space="PSUM") as pv_ps,
        ):
            # ---- persistent SBUF tensors ----
            wq_sb = cpool.tile([P, DC, EL], BF16, tag="wq")
            wk_sb = cpool.tile([P, DC, EL], BF16, tag="wk")
            wv_sb = cpool.tile([P, DC, EL], BF16, tag="wv")
            wout_sb = cpool.tile([P, EL // P, D], BF16, tag="wout")
            mask_sb = cpool.tile([P, P], BF16, tag="mask")
            mask2_sb = cpool.tile([P, 2, P], BF16, tag="mask2")
            qT_sb = cpool.tile([P, EL // P, L], BF16, tag="qT")
            kT_sb = cpool.tile([P, EL // P, L], BF16, tag="kT")
            vext_sb = cpool.tile([P, L // P, NH, Dh + 1], BF16, tag="vext")
            oT_sb = cpool.tile([P, EL // P, L], BF16, tag="oT")
            ones_f32 = cpool.tile([P, P], F32, tag="onesf")
            ones_sb = cpool.tile([P, P], F32R, tag="ones")

            # DMA order matters at startup: the first QKV matmul group needs
            # wq + the first xT chunk; everything else can trickle in behind
            xT_r = xT.ap().rearrange("(o p) l -> p o l", p=P)
            wq_r = wq.ap().rearrange("(o p) e -> p o e", p=P)
            xt0 = xpool.tile([P, DC, NL], BF16, tag="xt", name="xt0")
            for dc in range(0, DC, 2):
                nc.sync.dma_start(wq_sb[:, dc:dc + 2, :], wq_r[:, dc:dc + 2, :])
                nc.sync.dma_start(xt0[:, dc:dc + 2, :], xT_r[:, dc:dc + 2, 0:NL])
            nc.sync.dma_start(
                wk_sb[:], wk.ap().rearrange("(o p) e -> p o e", p=P))
            nc.sync.dma_start(
                wv_sb[:], wv.ap().rearrange("(o p) e -> p o e", p=P))
            nc.sync.dma_start(
                wout_sb[:], wout.ap().rearrange("(o p) d -> p o d", p=P))
            nc.sync.dma_start(mask_sb[:], masks[:, :])

            nc.vector.memset(ones_f32[:], 1.0)
            nc.vector.tensor_copy(out=ones_sb[:], in_=ones_f32[:])
            nc.vector.tensor_copy(out=mask2_sb[:, 0, :], in_=mask_sb[:, :])
            nc.vector.tensor_copy(out=mask2_sb[:, 1, :], in_=mask_sb[:, :])
            # ones column of vext (the softmax-denominator row of P@V)
            nc.vector.tensor_copy(
                out=vext_sb[:, :, :, Dh],
                in_=ones_f32[:, 0:L // P * NH].rearrange("p (a b) -> p a b", a=L // P),
            )

            def emit_qkv(lc):
                if lc == 0:
                    xt = xt0
                else:
                    xt = xpool.tile([P, DC, NL], BF16, tag="xt",
                                    name=f"xt{lc}")
                    for dc in range(0, DC, 2):
                        nc.sync.dma_start(
                            xt[:, dc:dc + 2, :],
                            xT_r[:, dc:dc + 2, lc * NL:(lc + 1) * NL])

                for w_sb, dst in ((wq_sb, qT_sb), (wk_sb, kT_sb)):
                    for ec in range(EL // P):
                        ps = mm_ps.tile([P, NQ], F32, tag="mm",
                                        name=f"qk_{lc}_{ec}")
                        for dc in range(DC):
                            nc.tensor.matmul(
                                ps[:],
                                w_sb[:, dc, ec * P:(ec + 1) * P],
                                xt[:, dc, :],
                                start=(dc == 0), stop=(dc == DC - 1),
                            )
                        nc.vector.tensor_copy(
                            out=dst[:, ec, lc * NL:(lc + 1) * NL], in_=ps[:])

                for lt in range(NL // P):
                    lo = lc * (NL // P) + lt
                    ps = mm_ps.tile([P, EL], F32, tag="mm",
                                    name=f"v_{lc}_{lt}")
                    for dc in range(DC):
                        nc.tensor.matmul(
                            ps[:],
                            xt[:, dc, lt * P:(lt + 1) * P],
                            wv_sb[:, dc, :],
                            start=(dc == 0), stop=(dc == DC - 1),
                        )
                    nc.vector.tensor_copy(
                        out=vext_sb[:, lo, :, 0:Dh],
                        in_=ps[:].rearrange("p (h e) -> p h e", h=NH),
                    )

                # previous phase's deferred reciprocal stages and norm
                # back-halves go after the v chains: the v chains cover the
                # qk-evacuation boundary, and by now the bounce DMAs have
                # long finished so nothing parks on them
                if lc >= 1:
                    emit_recip_stage_b(lc - 1, 0)
                    emit_recip_stage_b(lc - 1, 1)
                    for h in range(NH):
                        emit_norm_back_h(lc - 1, h)

            norm_state = {}        # (qc, h) -> (ot_un, rr)
            recip_pend = {}        # (qc, pair) -> state for recip stage B

            def emit_attn_pair(qc, pair, fillers=(), start_ki=4):
                nk = 4 * (qc + 1)          # causal k-chunks of 128
                heads = (2 * pair, 2 * pair + 1)
                pts = {}               # (h, ki) -> (pt AP [P, NQ], j)
                pvs = {}               # h -> accumulating PSUM tile
                fill_iter = iter(fillers)

                def emit_pv(h, ki):
                    ap, j = pts.pop((h, ki))
                    lo = P * j if j > 0 else 0
                    nc.tensor.matmul(
                        pvs[h][:, lo:],
                        vext_sb[:, ki, h, :],
                        ap[:, lo:],
                        start=(ki == 0), stop=(ki == nk - 1),
                        skip_group_check=(lo > 0 or ki == nk - 1),
                    )

                for ki in range(nk):
                    j = ki - 4 * qc    # >=0 on diagonal-crossing tiles
                    lo = P * j if j > 0 else 0
                    # both heads' score tiles share one 2-bank PSUM tile
                    # so a single EXP covers the pair
                    stp = st_ps.tile([P, 2, NQ], F32, tag="st",
                                     name=f"st_{qc}_{pair}_{ki}")
                    ptp = ptpool.tile([P, 2, NQ], BF16, tag="pt",
                                      name=f"pt_{qc}_{pair}_{ki}")
                    for idx, h in enumerate(heads):
                        hp = (h % 2) * 64
                        ec = h // 2
                        nc.tensor.matmul(
                            stp[:, idx, lo:],
                            kT_sb[hp:hp + 64, ec, ki * P:(ki + 1) * P],
                            qT_sb[hp:hp + 64, ec,
                                  qc * NQ + lo:(qc + 1) * NQ],
                            start=True, stop=True,
                        )
                        pts[(h, ki)] = (ptp[:, idx, :], j)
                    nc.scalar.activation(
                        out=ptp[:, :, lo:], in_=stp[:, :, lo:],
                        func=EXP, scale=scale)
                    if j >= 0:
                        # triangular mask on the diagonal 128-block; bf16
                        # SBUF-to-SBUF tensor_tensor runs 2x-packed on DVE
                        for idx in range(2):
                            nc.vector.tensor_tensor(
                                out=ptp[:, idx, P * j:P * (j + 1)],
                                in0=ptp[:, idx, P * j:P * (j + 1)],
                                in1=mask_sb[:, :],
                                op=MULT)
                    # P@V runs one ki behind the scores so the in-order
                    # PE stream never waits on the exp of the current ki
                    if ki >= 1:
                        for h in heads:
                            if ki == 1:
                                pvs[h] = pv_ps.tile([Dh + 1, NQ], F32,
                                                    name=f"po_{qc}_{h}",
                                                    tag="pv")
                            emit_pv(h, ki - 1)
                    # filler work (projection matmuls of the previous
                    # phase) absorbs the ScalarE exp pacing gap
                    if ki >= start_ki:
                        step = next(fill_iter, None)
                        if step is not None:
                            step()
                for h in heads:
                    emit_pv(h, nk - 1)

                # norm front stage A: evacuate both heads' PSUM and kick
                # off the DRAM-bounce DMAs that spread the [1,512] sum row
                # over 64 partitions. eng="split" routes one head's chain
                # through ScalarE/GpSimd so two chains run in parallel
                # (used for the very last pair).
                eng = "split" if (qc == QC - 1 and pair == 1) else "dve"
                fr = {}
                for n_h, h in enumerate(heads):
                    po = pvs[h]
                    alt = eng == "split" and n_h == 1
                    ot_un = npool.tile([64, NQ], F32, tag="otun",
                                       name=f"otun_{qc}_{h}")
                    rr = npool.tile([P, NQ], F32R, tag="rr",
                                    name=f"rr_{qc}_{h}")
                    rsum = npool.tile([P, NQ], F32, tag="rsum",
                                      name=f"rsum_{qc}_{h}")
                    if alt:
                        nc.scalar.copy(out=ot_un[:], in_=po[0:64, :])
                        nc.scalar.copy(out=rsum[64:65, :],
                                       in_=po[64:65, :])
                    else:
                        nc.vector.tensor_copy(out=ot_un[:], in_=po[0:64, :])
                        nc.vector.tensor_copy(out=rsum[64:65, :],
                                              in_=po[64:65, :])
                    fr[h] = (ot_un, rr, rsum, alt)
                    norm_state[(qc, h)] = (ot_un, rr)
                r64s = {}
                for h in heads:
                    _, _, rsum, alt = fr[h]
                    dq = nc.gpsimd if alt else nc.sync
                    dr1 = dpool.tile([NQ], F32,
                                     name=f"dr1_{qc}_{h}", tag="dr1")
                    dq.dma_start(
                        dr1[:].rearrange("(a b) -> a b", a=1),
                        rsum[64:65, :])
                    r64 = npool.tile([64, NQ // 64], F32, tag="r64",
                                     name=f"r64_{qc}_{h}")
                    dq.dma_start(
                        r64[:],
                        dr1[:].rearrange("(a b) -> a b", b=NQ // 64))
                    r64s[h] = r64
                recip_pend[(qc, pair)] = (heads, fr, r64s)

                # drain any filler work the ki-loop didn't consume
                for step in fill_iter:
                    step()

            def emit_recip_stage_b(qc, pair):
                # deferred: the approx reciprocals, the bounce-back DMAs and
                # the f32r casts. Deferring past the next phase's qkv
                # evacuations keeps the DVE queue from parking on the
                # bounce DMAs.
                heads, fr, r64s = recip_pend.pop((qc, pair))
                for h in heads:
                    r64b = npool.tile([64, NQ // 64], F32, tag="r64b",
                                      name=f"r64b_{qc}_{h}")
                    nc.vector.reciprocal_approx_fast(out=r64b[:],
                                                     in_=r64s[h][:])
                    r64s[h] = r64b
                rrfs = {}
                for h in heads:
                    _, _, _, alt = fr[h]
                    dq = nc.gpsimd if alt else nc.sync
                    dr2 = dpool.tile([NQ], F32,
                                     name=f"dr2_{qc}_{h}", tag="dr2")
                    dq.dma_start(
                        dr2[:].rearrange("(a b) -> a b", b=NQ // 64),
                        r64s[h][:])
                    rr_f = npool.tile([P, NQ], F32, tag="rrf",
                                      name=f"rrf_{qc}_{h}")
                    dq.dma_start(
                        rr_f[64:65, :],
                        dr2[:].rearrange("(a b) -> a b", a=1))
                    rrfs[h] = rr_f
                for h in heads:
                    _, rr, _, alt = fr[h]
                    if alt:
                        nc.scalar.copy(out=rr[64:65, :],
                                       in_=rrfs[h][64:65, :])
                    else:
                        nc.vector.tensor_copy(out=rr[64:65, :],
                                              in_=rrfs[h][64:65, :])

            def emit_norm_back_h(qc, h, mode="gp"):
                # back half: broadcast the reciprocal row via a K=1 matmul,
                # scale, DMA into oT (cross-partition move for the odd
                # half-heads). mode picks the engines so two epilogue
                # chains can run in parallel:
                #   "gp":  rs_sb on DVE, scale on GpSimd, DMA on gpsimd
                #   "dve": everything on DVE + sync (short latency)
                #   "alt": rs_sb on ScalarE, scale on GpSimd, DMA gpsimd
                hp = (h % 2) * 64
                ec = h // 2
                ot_un, rr = norm_state.pop((qc, h))
                ps_bc = pv_ps.tile([64, NQ], F32, tag="pv",
                                   name=f"bc_{qc}_{h}")
                nc.tensor.matmul(ps_bc[:], ones_sb[64:65, 0:64],
                                 rr[64:65, :], start=True, stop=True)
                rs_sb = wpool.tile([64, NQ], F32, tag="rs")
                tmp = wpool.tile([64, NQ], BF16, tag="tmp")
                if mode == "dve":
                    nc.vector.tensor_copy(out=rs_sb[:], in_=ps_bc[:])
                    nc.vector.tensor_tensor(out=tmp[:], in0=ot_un[:],
                                            in1=rs_sb[:], op=MULT)
                    nc.sync.dma_start(
                        oT_sb[hp:hp + 64, ec, qc * NQ:(qc + 1) * NQ],
                        tmp[:])
                else:
                    if mode == "alt":
                        nc.scalar.copy(out=rs_sb[:], in_=ps_bc[:])
                    else:
                        nc.vector.tensor_copy(out=rs_sb[:], in_=ps_bc[:])
                    nc.gpsimd.tensor_tensor(out=tmp[:], in0=ot_un[:],
                                            in1=rs_sb[:], op=MULT)
                    nc.gpsimd.dma_start(
                        oT_sb[hp:hp + 64, ec, qc * NQ:(qc + 1) * NQ],
                        tmp[:])

            def emit_proj_lt(lt, dve_only=False):
                # y = oT^T @ wout (partial over heads) for this l-chunk's rows
                y_sb = wpool.tile([P, 2, NQ], F32, tag="y")
                pss = []
                for do in range(D // NQ):
                    ps = mm_ps.tile([P, NQ], F32, tag="mm",
                                    name=f"y_{lt}_{do}")
                    for ec in range(EL // P):
                        nc.tensor.matmul(
                            ps[:],
                            oT_sb[:, ec, lt * P:(lt + 1) * P],
                            wout_sb[:, ec, do * NQ:(do + 1) * NQ],
                            start=(ec == 0), stop=(ec == EL // P - 1),
                        )
                    pss.append(ps)
                # evacuate the two halves on different engines in parallel
                if dve_only:
                    nc.vector.tensor_copy(out=y_sb[:, 0, :], in_=pss[0][:])
                else:
                    nc.scalar.copy(out=y_sb[:, 0, :], in_=pss[0][:])
                nc.vector.tensor_copy(out=y_sb[:, 1, :], in_=pss[1][:])
                nc.sync.dma_start(
                    out.ap()[lt * P:(lt + 1) * P, :].rearrange(
                        "p (a b) -> p a b", a=2),
                    y_sb[:])

            def proj_fillers(lts):
                # one closure per PE instruction (plus a no-PE evacuation
                # closure per l-chunk) so projection work can interleave
                # into the attention ki-loop; evacuations stay off ScalarE,
                # which paces the attention exps
                steps = []
                for lt in lts:
                    state = {}

                    def mk_mm(lt, do, ec, state=None):
                        def f(state=state, lt=lt, do=do, ec=ec):
                            if ec == 0:
                                state[do] = mm_ps.tile(
                                    [P, NQ], F32, tag="mm",
                                    name=f"y_{lt}_{do}")
                            nc.tensor.matmul(
                                state[do][:],
                                oT_sb[:, ec, lt * P:(lt + 1) * P],
                                wout_sb[:, ec, do * NQ:(do + 1) * NQ],
                                start=(ec == 0), stop=(ec == EL // P - 1),
                            )
                        return f

                    def mk_evac(lt, state=None):
                        def f(state=state, lt=lt):
                            y_sb = wpool.tile([P, 2, NQ], F32, tag="y")
                            nc.vector.tensor_copy(out=y_sb[:, 0, :],
                                                  in_=state[0][:])
                            nc.vector.tensor_copy(out=y_sb[:, 1, :],
                                                  in_=state[1][:])
                            nc.sync.dma_start(
                                out.ap()[lt * P:(lt + 1) * P, :].rearrange(
                                    "p (a b) -> p a b", a=2),
                                y_sb[:])
                        return f

                    for do in range(D // NQ):
                        for ec in range(EL // P):
                            steps.append(mk_mm(lt, do, ec, state=state))
                    steps.append(mk_evac(lt, state=state))
                return steps

            def emit_proj_lt_ecsplit(lt, phase):
                # epilogue helper: ec=0 only needs heads 0/1 in oT, ec=1
                # needs heads 2/3 — lets projection start before the last
                # pair's norm chains finish. The four concurrent
                # accumulators are spread over the st/mm/pv pools (8 banks).
                if phase == 0:
                    yp = st_ps.tile([P, 2, NQ], F32, tag="st",
                                    name=f"yps_{lt}")
                    pss = [yp[:, 0, :], yp[:, 1, :]]
                    _ec_state[lt] = pss
                    for do in range(D // NQ):
                        nc.tensor.matmul(
                            pss[do],
                            oT_sb[:, 0, lt * P:(lt + 1) * P],
                            wout_sb[:, 0, do * NQ:(do + 1) * NQ],
                            start=True, stop=False,
                        )
                else:
                    pss = _ec_state.pop(lt)
                    for do in range(D // NQ):
                        nc.tensor.matmul(
                            pss[do],
                            oT_sb[:, 1, lt * P:(lt + 1) * P],
                            wout_sb[:, 1, do * NQ:(do + 1) * NQ],
                            start=False, stop=True,
                        )
                    y_sb = wpool.tile([P, 2, NQ], F32, tag="y")
                    nc.scalar.copy(out=y_sb[:, 0, :], in_=pss[0])
                    nc.vector.tensor_copy(out=y_sb[:, 1, :], in_=pss[1])
                    nc.sync.dma_start(
                        out.ap()[lt * P:(lt + 1) * P, :].rearrange(
                            "p (a b) -> p a b", a=2),
                        y_sb[:])

            _ec_state = {}

            # phase schedule: qkv(ph) (norm-backs of ph-1 interleaved after
            # the qk chains) | attn(ph,0) | proj(ph-1) first half |
            # attn(ph,1) | proj(ph-1) second half. Projections of the
            # previous phase fill the gaps between attention pairs.
            # main pipeline: projection matmuls of the previous phase are
            # sprinkled INTO the attention ki-loops as PE fillers, so the
            # PE has work whenever ScalarE's exp stream falls behind
            for ph in range(QC):
                emit_qkv(ph)
                if ph >= 1:
                    f0 = proj_fillers([4 * (ph - 1), 4 * (ph - 1) + 1])
                else:
                    f0 = []
                emit_attn_pair(ph, 0, fillers=f0, start_ki=4)
                if ph == QC - 1:
                    # pair0's deferred reciprocals run while attn(3,1)'s
                    # DVE queue is otherwise empty (no masks until ki=12)
                    emit_recip_stage_b(ph, 0)
                if ph >= 1:
                    lts1 = [4 * (ph - 1) + 2]
                    if ph < QC - 1:
                        lts1.append(4 * (ph - 1) + 3)
                    f1 = proj_fillers(lts1)
                else:
                    f1 = []
                emit_attn_pair(ph, 1, fillers=f1, start_ki=2)

            # epilogue: pair1's reciprocal stage B runs engine-split, then
            # pair0's norm-backs (their bounces finished during attn(3,1)),
            # with the held-back projections and ec-split projections
            # covering the tail chains
            ql = QC - 1
            emit_recip_stage_b(ql, 1)
            emit_norm_back_h(ql, 0, mode="dve")
            emit_norm_back_h(ql, 1, mode="dve")
            emit_proj_lt_ecsplit(12, 0)
            emit_proj_lt_ecsplit(13, 0)
            emit_norm_back_h(ql, 2, mode="dve")
            emit_norm_back_h(ql, 3, mode="alt")
            emit_proj_lt(11)
            emit_proj_lt_ecsplit(12, 1)
            emit_proj_lt_ecsplit(13, 1)
            emit_proj_lt(14)
            emit_proj_lt(15)

    nc.compile()
    return nc


def _host_masks():
    k = np.arange(P)[:, None]
    q = np.arange(P)[None, :]
    return (k <= q).astype(np.float32)


def _shard(x, Wq, Wk, Wv, Wout):
    import ml_dtypes
    bf16 = ml_dtypes.bfloat16
    masks = _host_masks()
    in_maps = []
    for c in range(NCORES):
        b, g = c // NH, c % NH
        hs = slice(g * NH, (g + 1) * NH)
        in_maps.append({
            "xT": np.ascontiguousarray(x[b].T).astype(bf16),
            "wq": np.ascontiguousarray(Wq[:, hs, :].reshape(D, EL)).astype(bf16),
            "wk": np.ascontiguousarray(Wk[:, hs, :].reshape(D, EL)).astype(bf16),
            "wv": np.ascontiguousarray(Wv[:, hs, :].reshape(D, EL)).astype(bf16),
            "wout": np.ascontiguousarray(Wout[hs].reshape(EL, D)).astype(bf16),
            "masks": masks.astype(bf16),
        })
    return in_maps


_NC_CACHE = None


def _get_nc():
    global _NC_CACHE
    if _NC_CACHE is None:
        _NC_CACHE = build()
    return _NC_CACHE


def run(x, Wq, Wk, Wv, Wout, trace=False):
    nc = _get_nc()
    in_maps = _shard(np.asarray(x), np.asarray(Wq), np.asarray(Wk),
                     np.asarray(Wv), np.asarray(Wout))
    res = run_bass_kernel_spmd(nc, in_maps, core_ids=list(range(NCORES)),
                               trace=trace)
    parts = [res.results[c]["out"] for c in range(NCORES)]
    full = np.stack([
        parts[0] + parts[1] + parts[2] + parts[3],
        parts[4] + parts[5] + parts[6] + parts[7],
    ]).astype(np.float32)
    return full, res


def kernel(x, Wq, Wk, Wv, Wout):
    for _ in range(3):
        full, _ = run(x, Wq, Wk, Wv, Wout, trace=False)
        if np.isfinite(full).all():
            return full
    return full


# revision 36
# speedup vs baseline: 1.2430x; 1.1018x over previous
"""Causal multi-head attention (B=2, L=2048, D=1024, H=16, Dh=64) on 8 TRN2
NeuronCores.

Sharding: data-parallel over B (2 groups of 4 cores), tensor-parallel over H
within a group (4 heads per core). Each core computes QKV projections for its
heads, full causal attention per head (flash-style, scores kept transposed so
no on-chip transposes are needed), and a partial output projection
y_c = sum_h o_h @ Wout_h. The host sums the 4 partials per batch.

v2 restructure vs the original baseline:
  - Trapezoid streaming: scores + exp + P@V only touch columns right of the
    causal diagonal (per 128-wide k-tile), instead of memset-zeroing masked
    regions and streaming full 512-wide tiles.
  - Phase schedule: norm back-halves run at the top of the next phase (before
    attention, covering the qkv->attn dependency boundary); projections are
    split around attn(pair1); the epilogue splits the last projections by
    ec-half (ec0 only needs heads 0/1) to hide the final reciprocal chains.
  - Reciprocal of the softmax denominator: ScalarE Ln + Exp(scale=-1) for the
    last phase (2 instructions, low latency, same activation table set as the
    softmax Exp), DMA-bounce + reciprocal_approx_fast for earlier phases.
  - Engine balance: output-projection PSUM evacuations split ScalarE/DVE; the
    o*(1/sum) scale-mult runs on GpSimd; norm-path DMAs ride the gpsimd queue.
"""

import numpy as np

import concourse.bass as bass
import concourse.mybir as mybir
import concourse.tile as tile
from concourse import bacc
from concourse.bass_utils import run_bass_kernel_spmd

F32 = mybir.dt.float32
F32R = mybir.dt.float32r
BF16 = mybir.dt.bfloat16
EXP = mybir.ActivationFunctionType.Exp
MULT = mybir.AluOpType.mult

B, L, D, H = 2, 2048, 1024, 16
Dh = D // H
NCORES = 8
NH = 4            # heads per core
EL = NH * Dh      # local head dims = 256
P = 128
NQ = 512          # q-chunk width (scores free dim)
QC = L // NQ      # 4 q-chunks
DC = D // P       # 8 contraction chunks for projections
LC = 4            # xT l-chunks for QKV
NL = L // LC      # 512


def build():
    nc = bacc.Bacc("TRN2", target_bir_lowering=False, debug=False,
                   num_devices=NCORES)

    xT = nc.dram_tensor("xT", [D, L], BF16, kind="ExternalInput")
    wq = nc.dram_tensor("wq", [D, EL], BF16, kind="ExternalInput")
    wk = nc.dram_tensor("wk", [D, EL], BF16, kind="ExternalInput")
    wv = nc.dram_tensor("wv", [D, EL], BF16, kind="ExternalInput")
    wout = nc.dram_tensor("wout", [EL, D], BF16, kind="ExternalInput")
    masks = nc.dram_tensor("masks", [P, P], BF16, kind="ExternalInput")
    out = nc.dram_tensor("out", [L, D], F32, kind="ExternalOutput")

    scale = 1.0 / np.sqrt(Dh)

    with tile.TileContext(nc) as tc:
        with (
            tc.tile_pool(name="const", bufs=1) as cpool,
            tc.tile_pool(name="xt", bufs=2) as xpool,
            tc.tile_pool(name="pt", bufs=6) as ptpool,
            tc.tile_pool(name="work", bufs=3) as wpool,
            tc.tile_pool(name="norm", bufs=8) as npool,
            tc.tile_pool(name="dram", bufs=8, space="DRAM") as dpool,
            tc.tile_pool(name="mm", bufs=2, space="PSUM") as mm_ps,
            tc.tile_pool(name="st", bufs=2, space="PSUM") as st_ps,
            tc.tile_pool(name="pv", bufs=2, space="PSUM") as pv_ps,
        ):
            # ---- persistent SBUF tensors ----
            wq_sb = cpool.tile([P, DC, EL], BF16, tag="wq")
            wk_sb = cpool.tile([P, DC, EL], BF16, tag="wk")
            wv_sb = cpool.tile([P, DC, EL], BF16, tag="wv")
            wout_sb = cpool.tile([P, EL // P, D], BF16, tag="wout")
            mask_sb = cpool.tile([P, P], BF16, tag="mask")
            mask2_sb = cpool.tile([P, 2, P], BF16, tag="mask2")
            qT_sb = cpool.tile([P, EL // P, L], BF16, tag="qT")
            kT_sb = cpool.tile([P, EL // P, L], BF16, tag="kT")
            vext_sb = cpool.tile([P, L // P, NH, Dh + 1], BF16, tag="vext")
            oT_sb = cpool.tile([P, EL // P, L], BF16, tag="oT")
            ones_f32 = cpool.tile([P, P], F32, tag="onesf")
            ones_sb = cpool.tile([P, P], F32R, tag="ones")

            # DMA order matters at startup: the first QKV matmul group needs
            # wq + the first xT chunk; everything else can trickle in behind
            xT_r = xT.ap().rearrange("(o p) l -> p o l", p=P)
            wq_r = wq.ap().rearrange("(o p) e -> p o e", p=P)
            xt0 = xpool.tile([P, DC, NL], BF16, tag="xt", name="xt0")
            for dc in range(0, DC, 2):
                nc.sync.dma_start(wq_sb[:, dc:dc + 2, :], wq_r[:, dc:dc + 2, :])
                nc.sync.dma_start(xt0[:, dc:dc + 2, :], xT_r[:, dc:dc + 2, 0:NL])
            nc.sync.dma_start(
                wk_sb[:], wk.ap().rearrange("(o p) e -> p o e", p=P))
            nc.sync.dma_start(
                wv_sb[:], wv.ap().rearrange("(o p) e -> p o e", p=P))
            nc.sync.dma_start(
                wout_sb[:], wout.ap().rearrange("(o p) d -> p o d", p=P))
            nc.sync.dma_start(mask_sb[:], masks[:, :])

            nc.vector.memset(ones_f32[:], 1.0)
            nc.vector.tensor_copy(out=ones_sb[:], in_=ones_f32[:])
            nc.vector.tensor_copy(out=mask2_sb[:, 0, :], in_=mask_sb[:, :])
            nc.vector.tensor_copy(out=mask2_sb[:, 1, :], in_=mask_sb[:, :])
            # ones column of vext (the softmax-denominator row of P@V)
            nc.vector.tensor_copy(
                out=vext_sb[:, :, :, Dh],
                in_=ones_f32[:, 0:L // P * NH].rearrange("p (a b) -> p a b", a=L // P),
            )

            def emit_qkv(lc):
                if lc == 0:
                    xt = xt0
                else:
                    xt = xpool.tile([P, DC, NL], BF16, tag="xt",
                                    name=f"xt{lc}")
                    for dc in range(0, DC, 2):
                        nc.sync.dma_start(
                            xt[:, dc:dc + 2, :],
                            xT_r[:, dc:dc + 2, lc * NL:(lc + 1) * NL])

                for w_sb, dst in ((wq_sb, qT_sb), (wk_sb, kT_sb)):
                    for ec in range(EL // P):
                        ps = mm_ps.tile([P, NQ], F32, tag="mm",
                                        name=f"qk_{lc}_{ec}")
                        for dc in range(DC):
                            nc.tensor.matmul(
                                ps[:],
                                w_sb[:, dc, ec * P:(ec + 1) * P],
                                xt[:, dc, :],
                                start=(dc == 0), stop=(dc == DC - 1),
                            )
                        nc.vector.tensor_copy(
                            out=dst[:, ec, lc * NL:(lc + 1) * NL], in_=ps[:])

                for lt in range(NL // P):
                    lo = lc * (NL // P) + lt
                    ps = mm_ps.tile([P, EL], F32, tag="mm",
                                    name=f"v_{lc}_{lt}")
                    for dc in range(DC):
                        nc.tensor.matmul(
                            ps[:],
                            xt[:, dc, lt * P:(lt + 1) * P],
                            wv_sb[:, dc, :],
                            start=(dc == 0), stop=(dc == DC - 1),
                        )
                    nc.vector.tensor_copy(
                        out=vext_sb[:, lo, :, 0:Dh],
                        in_=ps[:].rearrange("p (h e) -> p h e", h=NH),
                    )

                # previous phase's norm back-halves go after the v chains:
                # the v chains cover the qk-evacuation boundary
                if lc >= 1:
                    for h in range(NH):
                        emit_norm_back_h(lc - 1, h)

            norm_state = {}        # (qc, h) -> (ot_un, rr)
            recip_pend = {}        # (qc, pair) -> state for recip stage B

            def emit_attn_pair(qc, pair, fillers=(), start_ki=4,
                               rate=2, drain=True):
                nk = 4 * (qc + 1)          # causal k-chunks of 128
                heads = (2 * pair, 2 * pair + 1)
                pts = {}               # (h, ki) -> (pt AP [P, NQ], j)
                pvs = {}               # h -> accumulating PSUM tile
                fill_iter = iter(fillers)

                def emit_pv(h, ki):
                    ap, j = pts.pop((h, ki))
                    lo = P * j if j > 0 else 0
                    nc.tensor.matmul(
                        pvs[h][:, lo:],
                        vext_sb[:, ki, h, :],
                        ap[:, lo:],
                        start=(ki == 0), stop=(ki == nk - 1),
                        skip_group_check=(lo > 0 or ki == nk - 1),
                    )

                for ki in range(nk):
                    j = ki - 4 * qc    # >=0 on diagonal-crossing tiles
                    lo = P * j if j > 0 else 0
                    # both heads' score tiles share one 2-bank PSUM tile
                    # so a single EXP covers the pair
                    stp = st_ps.tile([P, 2, NQ], F32, tag="st",
                                     name=f"st_{qc}_{pair}_{ki}")
                    ptp = ptpool.tile([P, 2, NQ], BF16, tag="pt",
                                      name=f"pt_{qc}_{pair}_{ki}")
                    for idx, h in enumerate(heads):
                        hp = (h % 2) * 64
                        ec = h // 2
                        nc.tensor.matmul(
                            stp[:, idx, lo:],
                            kT_sb[hp:hp + 64, ec, ki * P:(ki + 1) * P],
                            qT_sb[hp:hp + 64, ec,
                                  qc * NQ + lo:(qc + 1) * NQ],
                            start=True, stop=True,
                        )
                        pts[(h, ki)] = (ptp[:, idx, :], j)
                    nc.scalar.activation(
                        out=ptp[:, :, lo:], in_=stp[:, :, lo:],
                        func=EXP, scale=scale)
                    if j >= 0:
                        # triangular mask on the diagonal 128-block; bf16
                        # SBUF-to-SBUF tensor_tensor runs 2x-packed on DVE
                        for idx in range(2):
                            nc.vector.tensor_tensor(
                                out=ptp[:, idx, P * j:P * (j + 1)],
                                in0=ptp[:, idx, P * j:P * (j + 1)],
                                in1=mask_sb[:, :],
                                op=MULT)
                    # P@V runs one ki behind the scores so the in-order
                    # PE stream never waits on the exp of the current ki
                    if ki >= 1:
                        for h in heads:
                            if ki == 1:
                                pvs[h] = pv_ps.tile([Dh + 1, NQ], F32,
                                                    name=f"po_{qc}_{h}",
                                                    tag="pv")
                            emit_pv(h, ki - 1)
                    # filler work (projection matmuls of the previous
                    # phase) absorbs the ScalarE exp pacing gap
                    if ki >= start_ki:
                        for _ in range(rate):
                            step = next(fill_iter, None)
                            if step is not None:
                                step()
                for h in heads:
                    emit_pv(h, nk - 1)

                # norm front: evacuate both heads' PSUM; the sum row is
                # cast to f32r so the norm-back can broadcast it with a
                # K=1 matmul and take the reciprocal AFTER the broadcast
                # (64 lanes) — no DRAM bounce needed
                for h in heads:
                    po = pvs[h]
                    ot_un = npool.tile([64, NQ], F32, tag="otun",
                                       name=f"otun_{qc}_{h}")
                    nc.vector.tensor_copy(out=ot_un[:], in_=po[0:64, :])
                    rsumr = npool.tile([P, NQ], F32R, tag="rsum",
                                       name=f"rsum_{qc}_{h}")
                    nc.vector.tensor_copy(out=rsumr[64:65, :],
                                          in_=po[64:65, :])
                    norm_state[(qc, h)] = (ot_un, rsumr)

                # drain any filler work the ki-loop didn't consume
                if drain:
                    for step in fill_iter:
                        step()
                return fill_iter

            def emit_norm_back_h(qc, h, mode="gp"):
                # back half: broadcast the f32r SUM row to 64 partitions
                # via a K=1 matmul, reciprocal AFTER the broadcast (64 DVE
                # lanes, straight from PSUM), scale, DMA into oT (the DMA
                # is the cross-partition move for the odd half-heads).
                # mode picks engines so two epilogue chains can overlap:
                #   "gp":  scale-mult on GpSimd, DMA on gpsimd
                #   "dve": scale-mult on DVE, DMA on sync (short latency)
                hp = (h % 2) * 64
                ec = h // 2
                ot_un, rsumr = norm_state.pop((qc, h))
                ps_bc = pv_ps.tile([64, NQ], F32, tag="pv",
                                   name=f"bc_{qc}_{h}")
                nc.tensor.matmul(ps_bc[:], ones_sb[64:65, 0:64],
                                 rsumr[64:65, :], start=True, stop=True)
                rs_sb = wpool.tile([64, NQ], F32, tag="rs")
                nc.vector.reciprocal_approx_fast(out=rs_sb[:],
                                                 in_=ps_bc[:])
                tmp = wpool.tile([64, NQ], BF16, tag="tmp")
                if mode == "dve":
                    nc.vector.tensor_tensor(out=tmp[:], in0=ot_un[:],
                                            in1=rs_sb[:], op=MULT)
                    nc.sync.dma_start(
                        oT_sb[hp:hp + 64, ec, qc * NQ:(qc + 1) * NQ],
                        tmp[:])
                else:
                    nc.gpsimd.tensor_tensor(out=tmp[:], in0=ot_un[:],
                                            in1=rs_sb[:], op=MULT)
                    nc.gpsimd.dma_start(
                        oT_sb[hp:hp + 64, ec, qc * NQ:(qc + 1) * NQ],
                        tmp[:])

            def emit_proj_lt(lt, dve_only=False):
                # y = oT^T @ wout (partial over heads) for this l-chunk's rows
                y_sb = wpool.tile([P, 2, NQ], F32, tag="y")
                pss = []
                for do in range(D // NQ):
                    ps = mm_ps.tile([P, NQ], F32, tag="mm",
                                    name=f"y_{lt}_{do}")
                    for ec in range(EL // P):
                        nc.tensor.matmul(
                            ps[:],
                            oT_sb[:, ec, lt * P:(lt + 1) * P],
                            wout_sb[:, ec, do * NQ:(do + 1) * NQ],
                            start=(ec == 0), stop=(ec == EL // P - 1),
                        )
                    pss.append(ps)
                # evacuate the two halves on different engines in parallel
                if dve_only:
                    nc.vector.tensor_copy(out=y_sb[:, 0, :], in_=pss[0][:])
                else:
                    nc.scalar.copy(out=y_sb[:, 0, :], in_=pss[0][:])
                nc.vector.tensor_copy(out=y_sb[:, 1, :], in_=pss[1][:])
                nc.sync.dma_start(
                    out.ap()[lt * P:(lt + 1) * P, :].rearrange(
                        "p (a b) -> p a b", a=2),
                    y_sb[:])

            def proj_fillers(lts):
                # one closure per PE instruction (plus a no-PE evacuation
                # closure per l-chunk) so projection work can interleave
                # into the attention ki-loop; evacuations stay off ScalarE,
                # which paces the attention exps
                steps = []
                for lt in lts:
                    state = {}

                    def mk_mm(lt, do, ec, state=None):
                        def f(state=state, lt=lt, do=do, ec=ec):
                            if ec == 0:
                                state[do] = mm_ps.tile(
                                    [P, NQ], F32, tag="mm",
                                    name=f"y_{lt}_{do}")
                            nc.tensor.matmul(
                                state[do][:],
                                oT_sb[:, ec, lt * P:(lt + 1) * P],
                                wout_sb[:, ec, do * NQ:(do + 1) * NQ],
                                start=(ec == 0), stop=(ec == EL // P - 1),
                            )
                        return f

                    def mk_evac(lt, state=None):
                        def f(state=state, lt=lt):
                            y_sb = wpool.tile([P, 2, NQ], F32, tag="y")
                            nc.vector.tensor_copy(out=y_sb[:, 0, :],
                                                  in_=state[0][:])
                            nc.vector.tensor_copy(out=y_sb[:, 1, :],
                                                  in_=state[1][:])
                            nc.sync.dma_start(
                                out.ap()[lt * P:(lt + 1) * P, :].rearrange(
                                    "p (a b) -> p a b", a=2),
                                y_sb[:])
                        return f

                    for do in range(D // NQ):
                        for ec in range(EL // P):
                            steps.append(mk_mm(lt, do, ec, state=state))
                    steps.append(mk_evac(lt, state=state))
                return steps

            def emit_proj_lt_ecsplit(lt, phase):
                # epilogue helper: ec=0 only needs heads 0/1 in oT, ec=1
                # needs heads 2/3 — lets projection start before the last
                # pair's norm chains finish. The four concurrent
                # accumulators are spread over the st/mm/pv pools (8 banks).
                if phase == 0:
                    yp = st_ps.tile([P, 2, NQ], F32, tag="st",
                                    name=f"yps_{lt}")
                    pss = [yp[:, 0, :], yp[:, 1, :]]
                    _ec_state[lt] = pss
                    for do in range(D // NQ):
                        nc.tensor.matmul(
                            pss[do],
                            oT_sb[:, 0, lt * P:(lt + 1) * P],
                            wout_sb[:, 0, do * NQ:(do + 1) * NQ],
                            start=True, stop=False,
                        )
                else:
                    pss = _ec_state.pop(lt)
                    for do in range(D // NQ):
                        nc.tensor.matmul(
                            pss[do],
                            oT_sb[:, 1, lt * P:(lt + 1) * P],
                            wout_sb[:, 1, do * NQ:(do + 1) * NQ],
                            start=False, stop=True,
                        )
                    y_sb = wpool.tile([P, 2, NQ], F32, tag="y")
                    nc.scalar.copy(out=y_sb[:, 0, :], in_=pss[0])
                    nc.vector.tensor_copy(out=y_sb[:, 1, :], in_=pss[1])
                    nc.sync.dma_start(
                        out.ap()[lt * P:(lt + 1) * P, :].rearrange(
                            "p (a b) -> p a b", a=2),
                        y_sb[:])

            _ec_state = {}

            # phase schedule: qkv(ph) (norm-backs of ph-1 interleaved after
            # the qk chains) | attn(ph,0) | proj(ph-1) first half |
            # attn(ph,1) | proj(ph-1) second half. Projections of the
            # previous phase fill the gaps between attention pairs.
            # main pipeline: projection matmuls of the previous phase are
            # sprinkled INTO the attention ki-loops as PE fillers, so the
            # PE has work whenever ScalarE's exp stream falls behind; one
            # filler list flows across both pairs of a phase
            for ph in range(QC):
                emit_qkv(ph)
                if ph >= 1:
                    lts = [4 * (ph - 1) + k for k in range(4)]
                    if ph == QC - 1:
                        lts = lts[:-1]      # lt11 held back for the tail
                    fs = proj_fillers(lts)
                else:
                    fs = []
                rest = emit_attn_pair(ph, 0, fillers=fs, start_ki=4,
                                      rate=1, drain=False)
                emit_attn_pair(ph, 1, fillers=rest, start_ki=2,
                               rate=2, drain=True)

            # epilogue: pair0's norm-backs first, the ec0 halves of the
            # next projections (they only need heads 0/1) and the held-
            # back lt11 cover pair1's norm chains
            ql = QC - 1
            emit_norm_back_h(ql, 0, mode="dve")
            emit_norm_back_h(ql, 1, mode="gp")
            emit_proj_lt_ecsplit(12, 0)
            emit_proj_lt_ecsplit(13, 0)
            emit_norm_back_h(ql, 2, mode="dve")
            emit_norm_back_h(ql, 3, mode="gp")
            emit_proj_lt(11)
            emit_proj_lt_ecsplit(12, 1)
            emit_proj_lt_ecsplit(13, 1)
            emit_proj_lt(14)
            emit_proj_lt(15)

    nc.compile()
    return nc


def _host_masks():
    k = np.arange(P)[:, None]
    q = np.arange(P)[None, :]
    return (k <= q).astype(np.float32)


def _shard(x, Wq, Wk, Wv, Wout):
    import ml_dtypes
    bf16 = ml_dtypes.bfloat16
    masks = _host_masks()
    in_maps = []
    for c in range(NCORES):
        b, g = c // NH, c % NH
        hs = slice(g * NH, (g + 1) * NH)
        in_maps.append({
            "xT": np.ascontiguousarray(x[b].T).astype(bf16),
            "wq": np.ascontiguousarray(Wq[:, hs, :].reshape(D, EL)).astype(bf16),
            "wk": np.ascontiguousarray(Wk[:, hs, :].reshape(D, EL)).astype(bf16),
            "wv": np.ascontiguousarray(Wv[:, hs, :].reshape(D, EL)).astype(bf16),
            "wout": np.ascontiguousarray(Wout[hs].reshape(EL, D)).astype(bf16),
            "masks": masks.astype(bf16),
        })
    return in_maps


_NC_CACHE = None


def _get_nc():
    global _NC_CACHE
    if _NC_CACHE is None:
        _NC_CACHE = build()
    return _NC_CACHE


def run(x, Wq, Wk, Wv, Wout, trace=False):
    nc = _get_nc()
    in_maps = _shard(np.asarray(x), np.asarray(Wq), np.asarray(Wk),
                     np.asarray(Wv), np.asarray(Wout))
    res = run_bass_kernel_spmd(nc, in_maps, core_ids=list(range(NCORES)),
                               trace=trace)
    parts = [res.results[c]["out"] for c in range(NCORES)]
    full = np.stack([
        parts[0] + parts[1] + parts[2] + parts[3],
        parts[4] + parts[5] + parts[6] + parts[7],
    ]).astype(np.float32)
    return full, res


def kernel(x, Wq, Wk, Wv, Wout):
    for _ in range(3):
        full, _ = run(x, Wq, Wk, Wv, Wout, trace=False)
        if np.isfinite(full).all():
            return full
    return full
